# revision 1
# baseline (speedup 1.0000x reference)
"""BiLSTM-CRF loss kernel for Trainium2 (8 NeuronCores, data-parallel over batch).

Design (per core, B_loc=16 sequences):
  - All state kept transposed: hidden dim on partitions, batch on free dim.
  - LSTM recurrence: weights-stationary matmuls (8 gate-chunks x 2 K-tiles,
    N=16 batch streaming), per-step masking via copy_predicated with a
    DMA-broadcast mask-replica tile.
  - Input projection x @ W_ih^T computed on the fly in 32-step windows
    (embedding gather -> PE transpose -> N=512 matmuls), never hits DRAM.
  - Emissions computed incrementally (2 small matmuls per step/direction)
    into a (20, T*16) SBUF buffer.
  - CRF log-partition via the *backward* (beta) recursion in exp space,
    folded into the backward-LSTM phase step by step; periodic per-column
    rescaling (compensated in log space) keeps fp32 in range.
  - Gold-path score: unary via host-built one-hot mask x emit reduce;
    transition term via indirect row-gather of `transition` by tags.
"""

import numpy as np

PAD_IDX = 0
VOCAB, K, E, H = 30000, 20, 256, 256
B, T = 128, 512
NCORES = 8
BL = B // NCORES          # 16 sequences per core
WIN = 32                  # proj window (time steps)
NW = T // WIN             # 16 windows
RESCALE = 8               # CRF rescale interval

_cache = {}


def _build_program(dt_w):
    """Build the SPMD Bass program. dt_w: matmul weight/stream dtype."""
    from contextlib import ExitStack
    import concourse.bass as bass
    import concourse.bacc as bacc
    import concourse.tile as tile
    from concourse import mybir
    from concourse.masks import make_identity

    f32 = mybir.dt.float32
    i32 = mybir.dt.int32

    nc = bacc.Bacc(None, target_bir_lowering=False, debug=False)
    names = {}

    with ExitStack() as ctx:
        tc = ctx.enter_context(tile.TileContext(nc))
        dram = ctx.enter_context(tc.tile_pool(name="dram", bufs=1, space="DRAM"))

        def din(key, shape, dt=f32):
            t = dram.tile(shape, dt, kind="ExternalInput", name=key)
            names[key] = t.tensor.name
            return t

        emb = din("emb", [VOCAB, E])
        toks = din("toks", [T * BL, 1], i32)          # window-major token ids
        maskf = din("maskf", [1, T * BL])             # col = t*16+b
        masku = din("masku", [1, T * BL], mybir.dt.uint8)
        tags1h = din("tags1h", [K, T * BL], mybir.dt.uint8)  # one-hot(tag) * mask
        tagsnx = din("tagsnx", [T * BL, K], mybir.dt.uint8)  # shifted one-hot * mask
        tagsfl = din("tagsfl", [T * BL, 1], i32)      # tag ids, b-major
        wih = {d: din(f"wih_{d}", [E, 4 * H], dt_w) for d in "fb"}
        whh = {d: din(f"whh_{d}", [E, 4 * H], dt_w) for d in "fb"}
        bih = {d: din(f"bih_{d}", [128, 8]) for d in "fb"}
        woutT = din("woutT", [4, 128, K], dt_w)       # chunks: Fk0,Fk1,Bk0,Bk1
        bout = din("bout", [K, 1])
        transT = din("transT", [K, K])                # transition.T
        trans = din("trans", [K, K])                  # raw, for row gather
        out_loss = dram.tile([1, BL], f32, kind="ExternalOutput")
        names["out"] = out_loss.tensor.name

        sg = ctx.enter_context(tc.tile_pool(name="sg", bufs=1))       # singles
        tmp = ctx.enter_context(tc.tile_pool(name="tmp", bufs=4))     # step temps
        gat = ctx.enter_context(tc.tile_pool(name="gat", bufs=8))     # gather tiles
        winp = ctx.enter_context(tc.tile_pool(name="winp", bufs=2))   # xw windows
        xtw = ctx.enter_context(tc.tile_pool(name="xtw", bufs=3))
        fin = ctx.enter_context(tc.tile_pool(name="fin", bufs=3))     # finalize
        ps_g = ctx.enter_context(tc.tile_pool(name="ps_g", bufs=2, space="PSUM"))
        ps_w = ctx.enter_context(tc.tile_pool(name="ps_w", bufs=1, space="PSUM"))
        ps_t = ctx.enter_context(tc.tile_pool(name="ps_t", bufs=1, space="PSUM"))
        ps_s = ctx.enter_context(tc.tile_pool(name="ps_s", bufs=4, space="PSUM"))

        # ---- resident SBUF tensors ----
        s_wih = {d: sg.tile([128, 2, 4 * H], dt_w, tag=f"wih{d}", name=f"wih{d}") for d in "fb"}
        s_whh = {d: sg.tile([128, 2, 4 * H], dt_w, tag=f"whh{d}", name=f"whh{d}") for d in "fb"}
        for d in "fb":
            nc.sync.dma_start(out=s_wih[d][:], in_=wih[d][:].rearrange("(k p) m -> p k m", p=128))
            nc.sync.dma_start(out=s_whh[d][:], in_=whh[d][:].rearrange("(k p) m -> p k m", p=128))
        s_bih = {d: sg.tile([128, 8], f32, tag=f"bih{d}", name=f"bih{d}") for d in "fb"}
        for d in "fb":
            nc.sync.dma_start(out=s_bih[d][:], in_=bih[d][:])
        s_wout = sg.tile([128, 4, K], dt_w, tag="wout")
        nc.sync.dma_start(out=s_wout[:], in_=woutT[:].rearrange("c p k -> p c k"))
        s_bout = sg.tile([K, 1], f32, tag="bout")
        nc.sync.dma_start(out=s_bout[:], in_=bout[:])
        s_transT = sg.tile([K, K], f32, tag="transT")
        nc.sync.dma_start(out=s_transT[:], in_=transT[:])
        s_expAT = sg.tile([K, K], f32, tag="expAT")
        nc.scalar.activation(s_expAT[:], s_transT[:], mybir.ActivationFunctionType.Exp)

        ones = sg.tile([128, K], f32, tag="ones")
        nc.vector.memset(ones[:], 1.0)
        ident = sg.tile([128, 128], f32, tag="ident")
        make_identity(nc, ident[:])

        # mask replica: (128, T, BL), col = t*16+b, broadcast across partitions
        maskrep = sg.tile([128, T, BL], mybir.dt.uint8, tag="maskrep")
        nc.sync.dma_start(
            out=maskrep[:],
            in_=bass.AP(tensor=masku.tensor, offset=masku[:].offset,
                        ap=[[0, 128], [BL, T], [1, BL]]),
        )
        maskrow = sg.tile([1, T, BL], f32, tag="maskrow")
        nc.sync.dma_start(out=maskrow[:],
                          in_=bass.AP(tensor=maskf.tensor, offset=maskf[:].offset,
                                      ap=[[0, 1], [BL, T], [1, BL]]))

        emit = sg.tile([K, T, BL], f32, tag="emit")

        # all gather indices resident (one upfront DMA each)
        NT128 = T * BL // 128
        idxall = sg.tile([128, NT128], i32, tag="idxall")
        nc.sync.dma_start(out=idxall[:],
                          in_=bass.AP(tensor=toks.tensor, offset=toks[:].offset,
                                      ap=[[1, 128], [128, NT128]]))
        idxtag = sg.tile([128, NT128], i32, tag="idxtag")
        nc.sync.dma_start(out=idxtag[:],
                          in_=bass.AP(tensor=tagsfl.tensor, offset=tagsfl[:].offset,
                                      ap=[[1, 128], [128, NT128]]))
        s_t1h = sg.tile([K, T, BL], mybir.dt.uint8, tag="s_t1h")
        nc.sync.dma_start(out=s_t1h[:].rearrange("k t b -> k (t b)"), in_=tags1h[:])
        s_tnx = sg.tile([128, NT128, K], mybir.dt.uint8, tag="s_tnx")
        nc.sync.dma_start(out=s_tnx[:],
                          in_=tagsnx[:].rearrange("(n p) k -> p n k", p=128))

        # LSTM states (h in dt_w for matmul rhs, c in f32)
        st_h = {d: sg.tile([128, 2, BL], dt_w, tag=f"h{d}", name=f"h{d}") for d in "fb"}
        st_c = {d: sg.tile([128, 2, BL], f32, tag=f"c{d}", name=f"c{d}") for d in "fb"}
        for d in "fb":
            nc.vector.memset(st_h[d][:], 0.0)
            nc.vector.memset(st_c[d][:], 0.0)

        # CRF beta state (exp space) + log-scale accumulator
        Bv = sg.tile([K, BL], f32, tag="Bv")
        nc.vector.memset(Bv[:], 1.0)
        Lacc = sg.tile([1, BL], f32, tag="Lacc")
        nc.vector.memset(Lacc[:], 0.0)

        AF = mybir.ActivationFunctionType
        OP = mybir.AluOpType

        NG, GB = 2, BL // 2

        def mask_ap(t, parts, reps, g=None):
            """maskrep[:parts, t, cols] replicated reps times along a middle dim."""
            cs = slice(g * GB, (g + 1) * GB) if g is not None else slice(0, BL)
            base = maskrep[0:parts, t, cs]
            if reps == 1:
                return base
            return bass.AP(tensor=base.tensor, offset=base.offset,
                           ap=[base.ap[0], [0, reps], [1, cs.stop - cs.start]])

        def make_window(w, d):
            """Gather+transpose+project window w for direction d.
            Returns xw window tile (128, 8, BL, WIN) with bias folded."""
            xT = xtw.tile([128, 2, 512], dt_w, tag="xT")
            for g in range(4):
                j = w * 4 + g
                xg = gat.tile([128, E], f32, tag="xg")
                nc.gpsimd.indirect_dma_start(
                    out=xg[:], out_offset=None, in_=emb[:],
                    in_offset=bass.IndirectOffsetOnAxis(ap=idxall[:, j:j + 1], axis=0),
                )
                xg2 = gat.tile([128, E], f32, tag="xg2")
                nc.vector.tensor_copy(xg2[:], xg[:])
                for k in range(2):
                    pst = ps_t.tile([128, 128], f32, tag="pst")
                    nc.tensor.transpose(out=pst[:], in_=xg2[:, k * 128:(k + 1) * 128], identity=ident[:])
                    nc.vector.tensor_copy(xT[:, k, g * 128:(g + 1) * 128], pst[:])
            win = winp.tile([128, 8, BL, WIN], f32, tag=f"win{d}", name=f"win{d}")
            for m in range(8):
                psw = ps_w.tile([128, 512], f32, tag="psw")
                for k in range(2):
                    nc.tensor.matmul(psw[:], lhsT=s_wih[d][:, k, m * 128:(m + 1) * 128],
                                     rhs=xT[:, k, :], start=(k == 0), stop=(k == 1))
                nc.vector.tensor_scalar_add(win[:, m], psw[:], s_bih[d][:, m:m + 1])
            return win

        def lstm_mm(d, t):
            """Full-width recurrence matmuls (both groups share LDWEIGHTS)."""
            h = st_h[d]
            psg = ps_g.tile([128, 8, BL], f32, tag="psg", name="psg", bufs=2)
            for m in range(8):
                for k in range(2):
                    nc.tensor.matmul(psg[:, m], lhsT=s_whh[d][:, k, m * 128:(m + 1) * 128],
                                     rhs=h[:, k, :], start=(k == 0), stop=(k == 1))
            return psg

        def lstm_stepA(d, t, win, psg, g):
            """Gate add + activations for group g."""
            cs = slice(g * GB, (g + 1) * GB)
            toff = t % WIN
            gates = tmp.tile([128, 8, GB], f32, tag=f"gates{g}", name=f"gates{g}")
            nc.vector.tensor_tensor(gates[:], psg[:, :, cs], win[:, :, cs, toff], op=OP.add)
            gf = gates[:].rearrange("p m b -> p (m b)")
            nc.scalar.activation(gf[:, 0:4 * GB], gf[:, 0:4 * GB], AF.Sigmoid)
            nc.scalar.activation(gf[:, 4 * GB:6 * GB], gf[:, 4 * GB:6 * GB], AF.Tanh)
            nc.scalar.activation(gf[:, 6 * GB:8 * GB], gf[:, 6 * GB:8 * GB], AF.Sigmoid)
            return gates

        def lstm_stepB(d, t, gates, emit_mode, g):
            """Cell update for group g."""
            cs = slice(g * GB, (g + 1) * GB)
            h, c = st_h[d][:, :, cs], st_c[d][:, :, cs]
            gi, gff, gg, go = (gates[:, 0:2], gates[:, 2:4], gates[:, 4:6], gates[:, 6:8])
            cc = tmp.tile([128, 2, GB], f32, tag=f"cc{g}", name=f"cc{g}")
            ig = tmp.tile([128, 2, GB], f32, tag=f"ig{g}", name=f"ig{g}")
            nc.gpsimd.tensor_tensor(ig[:], gi, gg, op=OP.mult)
            nc.vector.tensor_tensor(cc[:], gff, c, op=OP.mult)
            nc.vector.tensor_tensor(cc[:], cc[:], ig[:], op=OP.add)
            m2 = mask_ap(t, 128, 2, g)
            nc.vector.copy_predicated(c, m2, cc[:])
            th = tmp.tile([128, 2, GB], f32, tag=f"th{g}", name=f"th{g}")
            nc.scalar.activation(th[:], cc[:], AF.Tanh)
            hh = tmp.tile([128, 2, GB], dt_w, tag=f"hh{g}", name=f"hh{g}")
            nc.vector.tensor_tensor(hh[:], go, th[:], op=OP.mult)
            nc.vector.copy_predicated(h, m2, hh[:])

        def emit_step(d, t, emit_mode):
            h = st_h[d]
            pse = ps_s.tile([K, BL], f32, tag="pssm", name="pse")
            cbase = 0 if d == "f" else 2
            for k in range(2):
                nc.tensor.matmul(pse[:], lhsT=s_wout[:, cbase + k, :], rhs=h[:, k, :],
                                 start=(k == 0), stop=(k == 1))
            if emit_mode == "f":
                nc.vector.tensor_scalar_add(emit[:, t, :], pse[:], s_bout[:, 0:1])
            else:
                nc.vector.tensor_tensor(emit[:, t, :], pse[:], emit[:, t, :], op=OP.add)

        # warm-up matmuls: make PE's clock pass every weight-producing op so
        # steady-state matmuls carry at most one semaphore wait
        for wt in [s_wih["f"][:, 0, 0:1], s_wih["b"][:, 0, 0:1],
                   s_whh["f"][:, 0, 0:1], s_whh["b"][:, 0, 0:1],
                   s_wout[:, 0, 0:1]]:
            psd = ps_s.tile([1, 1], f32, tag="pssm")
            nc.tensor.matmul(psd[:], lhsT=wt, rhs=wt, start=True, stop=True)
        psd = ps_s.tile([1, 1], f32, tag="pssm")
        nc.tensor.matmul(psd[:], lhsT=s_expAT[0:K, 0:1], rhs=s_expAT[0:K, 0:1], start=True, stop=True)
        psd = ps_s.tile([1, 1], f32, tag="pssm")
        nc.tensor.matmul(psd[:], lhsT=ident[:, 0:1], rhs=ident[:, 0:1], start=True, stop=True)

        # ---------------- forward phase ----------------
        for w in range(NW):
            win = make_window(w, "f")
            for t in range(w * WIN, (w + 1) * WIN):
                psg = lstm_mm("f", t)
                gts = [lstm_stepA("f", t, win, psg, g) for g in range(NG)]
                for g in range(NG):
                    lstm_stepB("f", t, gts[g], "f", g)
                emit_step("f", t, "f")

        # ---------------- backward phase + CRF beta ----------------
        expE_prev = [None, None]
        for w in range(NW - 1, -1, -1):
            win = make_window(w, "b")
            for t in range((w + 1) * WIN - 1, w * WIN - 1, -1):
                psg = lstm_mm("b", t)
                gts = [lstm_stepA("b", t, win, psg, g) for g in range(NG)]
                for g in range(NG):
                    lstm_stepB("b", t, gts[g], "b", g)
                emit_step("b", t, "b")
                for g in range(NG):
                    cs = slice(g * GB, (g + 1) * GB)
                    expE = tmp.tile([K, GB], f32, tag=f"expE{g}", name=f"expE{g}")
                    nc.scalar.activation(expE[:], emit[:, t, cs], AF.Exp)
                    if t < T - 1:
                        bp = tmp.tile([K, GB], f32, tag=f"bp{g}", name=f"bp{g}")
                        nc.vector.tensor_tensor(bp[:], Bv[:, cs], expE_prev[g][:], op=OP.mult)
                        psb = ps_s.tile([K, GB], f32, tag="pssm", name="psb")
                        nc.tensor.matmul(psb[:], lhsT=s_expAT[:], rhs=bp[:], start=True, stop=True)
                        nc.vector.copy_predicated(Bv[:, cs], mask_ap(t + 1, K, 1, g), psb[:])
                    expE_prev[g] = expE
                if t < T - 1 and t % RESCALE == 0 and t > 0:
                    pss = ps_s.tile([1, BL], f32, tag="pssm", name="pss")
                    nc.tensor.matmul(pss[:], lhsT=ones[0:K, 0:1], rhs=Bv[:], start=True, stop=True)
                    rr = tmp.tile([1, BL], f32, tag="rr")
                    nc.vector.reciprocal(rr[:], pss[:])
                    psr = ps_s.tile([K, BL], f32, tag="pssm", name="psr")
                    nc.tensor.matmul(psr[:], lhsT=ones[0:1, 0:K], rhs=rr[:], start=True, stop=True)
                    sc = tmp.tile([K, BL], f32, tag="sc")
                    nc.vector.tensor_tensor(sc[:], Bv[:], psr[:], op=OP.mult)
                    nc.vector.copy_predicated(Bv[:], mask_ap(t, K, 1), sc[:])
                    lns = tmp.tile([1, BL], f32, tag="lns")
                    nc.scalar.activation(lns[:], pss[:], AF.Ln)
                    nc.vector.tensor_tensor(lns[:], lns[:], maskrow[0:1, t, :], op=OP.mult)
                    nc.vector.tensor_tensor(Lacc[:], Lacc[:], lns[:], op=OP.add)

        # ---------------- finalize ----------------
        # log partition: logZ = ln(sum_i expE_0 * Bv_0) + Lacc
        zt = fin.tile([K, BL], f32, tag="zt")
        for g in range(NG):
            cs = slice(g * GB, (g + 1) * GB)
            nc.vector.tensor_tensor(zt[:, cs], Bv[:, cs], expE_prev[g][:], op=OP.mult)
        psz = ps_s.tile([1, BL], f32, tag="pssm")
        nc.tensor.matmul(psz[:], lhsT=ones[0:K, 0:1], rhs=zt[:], start=True, stop=True)
        logZ = fin.tile([1, BL], f32, tag="logZ")
        nc.scalar.activation(logZ[:], psz[:], AF.Ln)
        nc.vector.tensor_tensor(logZ[:], logZ[:], Lacc[:], op=OP.add)

        # unary gold score: sum over (j,t) of tags1h * emit, keep b
        Uacc = fin.tile([K, BL], f32, tag="Uacc")
        nc.vector.memset(Uacc[:], 0.0)
        CH = 32
        TC = T // CH
        for ci in range(CH):
            t1 = fin.tile([K, TC * BL], f32, tag="t1")
            nc.vector.tensor_copy(t1[:], s_t1h[:, ci * TC:(ci + 1) * TC, :].rearrange("p t b -> p (t b)"))
            um = fin.tile([K, TC * BL], f32, tag="um")
            nc.vector.tensor_tensor(
                um[:], t1[:], emit[:, ci * TC:(ci + 1) * TC, :].rearrange("p t b -> p (t b)"),
                op=OP.mult)
            ur = fin.tile([K, BL], f32, tag="ur")
            umr = bass.AP(tensor=um.tensor, offset=um[:].offset,
                          ap=[um[:].ap[0], [1, BL], [BL, TC]])
            nc.vector.tensor_reduce(ur[:], umr, axis=mybir.AxisListType.X, op=OP.add)
            nc.vector.tensor_tensor(Uacc[:], Uacc[:], ur[:], op=OP.add)
        psu = ps_s.tile([1, BL], f32, tag="pssm")
        nc.tensor.matmul(psu[:], lhsT=ones[0:K, 0:1], rhs=Uacc[:], start=True, stop=True)
        score = fin.tile([1, BL], f32, tag="score")
        nc.vector.tensor_copy(score[:], psu[:])

        # transition gold score via row gather
        QT = T // 128
        TRbuf = fin.tile([128, NT128], f32, tag="TRbuf")
        for i in range(NT128):
            tr = gat.tile([128, K], f32, tag="tr")
            nc.gpsimd.indirect_dma_start(
                out=tr[:], out_offset=None, in_=trans[:],
                in_offset=bass.IndirectOffsetOnAxis(ap=idxtag[:, i:i + 1], axis=0))
            sel = gat.tile([128, K], f32, tag="sel")
            nc.vector.tensor_copy(sel[:], s_tnx[:, i, :])
            nc.vector.tensor_tensor(tr[:], tr[:], sel[:], op=OP.mult)
            nc.vector.tensor_reduce(TRbuf[:, i:i + 1], tr[:], axis=mybir.AxisListType.X, op=OP.add)
        pstr = ps_s.tile([1, NT128], f32, tag="pssm")
        nc.tensor.matmul(pstr[:], lhsT=ones[:, 0:1], rhs=TRbuf[:], start=True, stop=True)
        trv = fin.tile([1, BL], f32, tag="trv")
        ptr_ap = bass.AP(tensor=pstr.tensor, offset=pstr[:].offset,
                         ap=[pstr[:].ap[0], [QT, BL], [1, QT]])
        nc.vector.tensor_reduce(trv[:], ptr_ap, axis=mybir.AxisListType.X, op=OP.add)

        # loss = logZ - (score + trans)
        nc.vector.tensor_tensor(score[:], score[:], trv[:], op=OP.add)
        res = fin.tile([1, BL], f32, tag="res")
        nc.vector.tensor_tensor(res[:], logZ[:], score[:], op=OP.subtract)
        nc.sync.dma_start(out=out_loss[:], in_=res[:])

    nc.compile()
    return nc, names


def _prep_core(inputs, k, dt_np):
    """Build the per-core input map (host-side index plumbing only)."""
    s = slice(k * BL, (k + 1) * BL)
    sent = np.asarray(inputs["sentences"][s])          # (16, 512) i32
    tags = np.asarray(inputs["tags"][s])               # (16, 512) i32
    mask = (sent != PAD_IDX)
    maskf = mask.T.astype(np.float32).reshape(1, T * BL)       # col=t*16+b
    toks = sent.reshape(BL, NW, WIN).transpose(1, 0, 2).reshape(T * BL, 1)
    oh = (tags[:, :, None] == np.arange(K)[None, None, :])
    tags1h = (oh & mask[:, :, None]).transpose(2, 1, 0).reshape(K, T * BL)
    tnx = np.zeros((BL, T, K), np.float32)
    tnx[:, :-1, :] = (oh[:, 1:, :] & mask[:, 1:, None]).astype(np.float32)
    m = {
        "toks": toks.astype(np.int32),
        "maskf": maskf,
        "masku": mask.T.astype(np.uint8).reshape(1, T * BL),
        "tags1h": tags1h.astype(np.uint8),
        "tagsnx": tnx.reshape(T * BL, K).astype(np.uint8),
        "tagsfl": tags.reshape(T * BL, 1).astype(np.int32),
        "emb": np.asarray(inputs["embedding"], np.float32),
        "wih_f": np.ascontiguousarray(np.asarray(inputs["w_ih_f"]).T).astype(dt_np),
        "wih_b": np.ascontiguousarray(np.asarray(inputs["w_ih_b"]).T).astype(dt_np),
        "whh_f": np.ascontiguousarray(np.asarray(inputs["w_hh_f"]).T).astype(dt_np),
        "whh_b": np.ascontiguousarray(np.asarray(inputs["w_hh_b"]).T).astype(dt_np),
        "bih_f": np.ascontiguousarray(np.asarray(inputs["b_f"]).reshape(8, 128).T).astype(np.float32),
        "bih_b": np.ascontiguousarray(np.asarray(inputs["b_b"]).reshape(8, 128).T).astype(np.float32),
        "woutT": np.ascontiguousarray(np.asarray(inputs["w_out"]).T.reshape(4, 128, K)).astype(dt_np),
        "bout": np.asarray(inputs["b_out"]).reshape(K, 1).astype(np.float32),
        "transT": np.ascontiguousarray(np.asarray(inputs["transition"]).T).astype(np.float32),
        "trans": np.asarray(inputs["transition"], np.float32),
    }
    return m


def kernel(**inputs):
    import ml_dtypes
    from concourse import mybir
    from concourse.bass_utils import run_bass_kernel_spmd

    use_bf16 = _cache.get("use_bf16", True)
    key = ("prog", use_bf16)
    if key not in _cache:
        dt_w = mybir.dt.bfloat16 if use_bf16 else mybir.dt.float32
        _cache[key] = _build_program(dt_w)
    nc, names = _cache[key]
    dt_np = ml_dtypes.bfloat16 if use_bf16 else np.float32

    in_maps = []
    for k in range(NCORES):
        m = _prep_core(inputs, k, dt_np)
        in_maps.append({names[kk]: vv for kk, vv in m.items()})

    res = run_bass_kernel_spmd(nc, in_maps, core_ids=list(range(NCORES)),
                               **_cache.get("run_kwargs", {}))
    out = np.concatenate([r[names["out"]].reshape(BL) for r in res.results])
    _cache["last_results"] = res
    return out.astype(np.float32)



# revision 11
# speedup vs baseline: 2.3259x; 2.3259x over previous
"""BiLSTM-CRF loss kernel for Trainium2 (8 NeuronCores, data-parallel over batch).

v2 design (per core, B_loc=16 sequences):
  - Forward and backward LSTM directions run INTERLEAVED in a single
    512-iteration loop (iter i: fwd step t=i, bwd step t=511-i) so the two
    independent recurrence chains fill each other's engine stalls.
  - Gate order host-permuted to (i, f, o, g) so activations are 2 instrs
    per step: sigmoid over 96 cols + tanh over 32 cols.
  - xw window injected into the gate PSUM via an identity matmul
    (start=True) before the 16 W_hh matmuls accumulate on top; the ACT
    engine reads gates straight from PSUM (no separate gate-add).
  - c update is unmasked (pad mask is a suffix per sequence; the unfrozen
    c is never read back and stays bounded), h masked via copy_predicated.
  - h history per 32-step window -> batched emission matmuls (4/window/dir)
    instead of 2 per step.
  - All exp() for the CRF batched per window in the epilogue: keeps the
    sigmoid+tanh activation tables resident all of phase 1 (no
    ACT_TABLE_LOAD thrash).
  - CRF log-partition via backward beta recursion in exp space as a
    separate 511-step phase; rescaling folded into the next step's expE
    slice (off the critical path), ln() of the scales deferred to one
    batched instruction at the end.
  - Gold-path score (unary + transition gather) interleaved into the beta
    phase.
"""

import numpy as np

PAD_IDX = 0
VOCAB, K, E, H = 30000, 20, 256, 256
B, T = 128, 512
NCORES = 8
BL = B // NCORES          # 16 sequences per core
WIN = 32                  # proj window (time steps)
NW = T // WIN             # 16 windows
RESCALE = 8               # CRF rescale interval

_cache = {}


def _build_program(dt_w):
    """Build the SPMD Bass program. dt_w: matmul weight/stream dtype."""
    from contextlib import ExitStack
    import concourse.bass as bass
    import concourse.bacc as bacc
    import concourse.tile as tile
    from concourse import mybir
    from concourse.masks import make_identity

    f32 = mybir.dt.float32
    i32 = mybir.dt.int32

    nc = bacc.Bacc(None, target_bir_lowering=False, debug=False)
    names = {}

    with ExitStack() as ctx:
        tc = ctx.enter_context(tile.TileContext(nc))
        dram = ctx.enter_context(tc.tile_pool(name="dram", bufs=1, space="DRAM"))

        def din(key, shape, dt=f32):
            t = dram.tile(shape, dt, kind="ExternalInput", name=key)
            names[key] = t.tensor.name
            return t

        emb = din("emb", [VOCAB, E], dt_w)
        toks = din("toks", [T * BL, 1], i32)          # (w, j, b) window/j-major
        masku = din("masku", [1, T * BL], mybir.dt.uint8)  # col = t*16+b
        tags1h = din("tags1h", [K, T * BL], mybir.dt.uint8)  # one-hot(tag) * mask
        tagsnx = din("tagsnx", [T * BL, K])           # shifted one-hot * mask, f32
        tagsfl = din("tagsfl", [T * BL, 1], i32)      # tag ids, b-major
        wih = {d: din(f"wih_{d}", [E, 4 * H], dt_w) for d in "fb"}
        whh = {d: din(f"whh_{d}", [E, 4 * H], dt_w) for d in "fb"}
        bih = {d: din(f"bih_{d}", [128, 8]) for d in "fb"}
        woutT = din("woutT", [4, 128, K], dt_w)       # chunks: Fk0,Fk1,Bk0,Bk1
        bout = din("bout", [K, 1])
        expAT = din("expAT", [K, K])                  # exp(transition).T
        trans = din("trans", [K, K])                  # raw, for row gather
        out_loss = dram.tile([1, BL], f32, kind="ExternalOutput")
        names["out"] = out_loss.tensor.name

        sg = ctx.enter_context(tc.tile_pool(name="sg", bufs=1))       # singles
        tmp = ctx.enter_context(tc.tile_pool(name="tmp", bufs=3))     # step temps
        gat = ctx.enter_context(tc.tile_pool(name="gat", bufs=4))     # gather tiles
        winp = ctx.enter_context(tc.tile_pool(name="winp", bufs=2))   # xw windows
        hhp = ctx.enter_context(tc.tile_pool(name="hhp", bufs=2))     # h history
        xtw = ctx.enter_context(tc.tile_pool(name="xtw", bufs=2))
        fin = ctx.enter_context(tc.tile_pool(name="fin", bufs=3))     # finalize
        ps_g = ctx.enter_context(tc.tile_pool(name="ps_g", bufs=2, space="PSUM"))
        ps_w = ctx.enter_context(tc.tile_pool(name="ps_w", bufs=2, space="PSUM"))
        ps_e = ctx.enter_context(tc.tile_pool(name="ps_e", bufs=2, space="PSUM"))
        ps_s = ctx.enter_context(tc.tile_pool(name="ps_s", bufs=2, space="PSUM"))

        # ---- resident SBUF tensors ----
        s_wih = {d: sg.tile([128, 2, 4 * H], dt_w, tag=f"wih{d}", name=f"wih{d}") for d in "fb"}
        s_whh = {d: sg.tile([128, 2, 4 * H], dt_w, tag=f"whh{d}", name=f"whh{d}") for d in "fb"}
        for d in "fb":
            nc.sync.dma_start(out=s_wih[d][:], in_=wih[d][:].rearrange("(k p) m -> p k m", p=128))
            nc.sync.dma_start(out=s_whh[d][:], in_=whh[d][:].rearrange("(k p) m -> p k m", p=128))
        s_bih = {d: sg.tile([128, 8], f32, tag=f"bih{d}", name=f"bih{d}") for d in "fb"}
        for d in "fb":
            nc.sync.dma_start(out=s_bih[d][:], in_=bih[d][:])
        s_wout = sg.tile([128, 4, K], dt_w, tag="wout")
        nc.sync.dma_start(out=s_wout[:], in_=woutT[:].rearrange("c p k -> p c k"))
        s_bout = sg.tile([K, 1], f32, tag="bout")
        nc.sync.dma_start(out=s_bout[:], in_=bout[:])
        s_expAT = sg.tile([K, K], f32, tag="expAT")
        nc.sync.dma_start(out=s_expAT[:], in_=expAT[:])

        ones = sg.tile([128, K], f32, tag="ones")
        nc.vector.memset(ones[:], 1.0)
        identb = sg.tile([128, 128], dt_w, tag="identb")
        make_identity(nc, identb[:])

        # mask replica: (128, T, BL), col = t*16+b, broadcast across partitions
        maskrep = sg.tile([128, T, BL], mybir.dt.uint8, tag="maskrep")
        nc.sync.dma_start(
            out=maskrep[:],
            in_=bass.AP(tensor=masku.tensor, offset=masku[:].offset,
                        ap=[[0, 128], [BL, T], [1, BL]]),
        )

        emit = sg.tile([K, T, BL], f32, tag="emit")
        expE = sg.tile([K, T, BL], f32, tag="expE")

        # gather indices resident (one upfront DMA each)
        NT128 = T * BL // 128
        idxall = sg.tile([128, NT128], i32, tag="idxall")
        nc.sync.dma_start(out=idxall[:],
                          in_=bass.AP(tensor=toks.tensor, offset=toks[:].offset,
                                      ap=[[1, 128], [128, NT128]]))
        idxtag = sg.tile([128, NT128], i32, tag="idxtag")
        nc.sync.dma_start(out=idxtag[:],
                          in_=bass.AP(tensor=tagsfl.tensor, offset=tagsfl[:].offset,
                                      ap=[[1, 128], [128, NT128]]))
        s_t1h = sg.tile([K, T, BL], mybir.dt.uint8, tag="s_t1h")
        nc.sync.dma_start(out=s_t1h[:].rearrange("k t b -> k (t b)"), in_=tags1h[:])
        s_tnx = sg.tile([128, NT128, K], f32, tag="s_tnx")
        nc.sync.dma_start(out=s_tnx[:],
                          in_=tagsnx[:].rearrange("(n p) k -> p n k", p=128))

        # LSTM states (h in dt_w for matmul rhs, c in f32)
        st_h = {d: sg.tile([128, 2, BL], dt_w, tag=f"h{d}", name=f"h{d}") for d in "fb"}
        st_c = {d: sg.tile([128, 2, BL], f32, tag=f"c{d}", name=f"c{d}") for d in "fb"}
        for d in "fb":
            nc.vector.memset(st_h[d][:], 0.0)
            nc.vector.memset(st_c[d][:], 0.0)

        # CRF beta state (exp space) + deferred-ln scale buffer
        Bv = sg.tile([K, BL], f32, tag="Bv")
        nc.vector.memset(Bv[:], 1.0)
        NRS = T // RESCALE
        sums = sg.tile([1, NRS, BL], f32, tag="sums")
        nc.vector.memset(sums[:], 1.0)

        AF = mybir.ActivationFunctionType
        OP = mybir.AluOpType

        def mask_ap(t, parts, reps):
            """maskrep[:parts, t, :] replicated reps times along a middle dim."""
            base = maskrep[0:parts, t, :]
            if reps == 1:
                return base
            return bass.AP(tensor=base.tensor, offset=base.offset,
                           ap=[base.ap[0], [0, reps], [1, BL]])

        # warm-up matmuls: make PE's clock pass every weight-producing op so
        # steady-state matmuls carry at most one semaphore wait
        for wt in [s_wih["f"][:, 0, 0:1], s_wih["b"][:, 0, 0:1],
                   s_whh["f"][:, 0, 0:1], s_whh["b"][:, 0, 0:1],
                   s_wout[:, 0, 0:1], identb[:, 0:1]]:
            psd = ps_s.tile([1, 1], f32, tag="pssm")
            nc.tensor.matmul(psd[:], lhsT=wt, rhs=wt, start=True, stop=True)
        psd = ps_s.tile([1, 1], f32, tag="pssm")
        nc.tensor.matmul(psd[:], lhsT=s_expAT[0:K, 0:1], rhs=s_expAT[0:K, 0:1], start=True, stop=True)
        psd = ps_s.tile([1, 1], f32, tag="pssm")
        nc.tensor.matmul(psd[:], lhsT=ones[0:1, 0:1], rhs=ones[0:1, 0:1], start=True, stop=True)

        # ---------- window machinery ----------
        # win layout: (128, WIN, 8, BL) -> inject rhs win[:, j, :, :] is one
        # contiguous 128-col slice.  h_hist: (128, 2, WIN, BL) (k, j, b).
        cur_win = {}
        cur_hist = {}

        def build_window_thunks(d, tw):
            """Return (win_tile, thunk list) building xw window for t-window tw."""
            win = winp.tile([128, WIN, 8, BL], dt_w, tag=f"win{d}", name=f"win{d}")
            xT = xtw.tile([128, 2, 512], dt_w, tag=f"xT{d}", name=f"xT{d}")
            thunks = []
            pst_box = {}

            def gather(g):
                xg = gat.tile([128, E], dt_w, tag=f"xg{d}", name=f"xg{d}")
                nc.gpsimd.indirect_dma_start(
                    out=xg[:], out_offset=None, in_=emb[:],
                    in_offset=bass.IndirectOffsetOnAxis(ap=idxall[:, tw * 4 + g:tw * 4 + g + 1], axis=0),
                )
                pst_box[g] = xg

            def tp(g, k):
                xg = pst_box[g]
                pst = ps_s.tile([128, 128], dt_w, tag="pssm", name="pst")
                nc.tensor.transpose(out=pst[:], in_=xg[:, k * 128:(k + 1) * 128], identity=identb[:])
                nc.vector.tensor_copy(xT[:, k, g * 128:(g + 1) * 128], pst[:])

            for g in range(4):
                thunks.append(lambda g=g: gather(g))
                for k in range(2):
                    thunks.append(lambda g=g, k=k: tp(g, k))

            def proj(m):
                psw = ps_w.tile([128, 512], f32, tag="psw", name="psw")
                for k in range(2):
                    nc.tensor.matmul(psw[:], lhsT=s_wih[d][:, k, m * 128:(m + 1) * 128],
                                     rhs=xT[:, k, :], start=(k == 0), stop=(k == 1))
                # psw cols are (j, b); win[:, :, m, :] has free dims (j, b)
                nc.vector.tensor_scalar_add(win[:, :, m, :], psw[:], s_bih[d][:, m:m + 1])

            for m in range(8):
                thunks.append(lambda m=m: proj(m))
            return win, thunks

        def new_hist(d):
            hist = hhp.tile([128, 2, WIN, BL], dt_w, tag=f"hist{d}", name=f"hist{d}")
            return hist

        def emit_window(d, tw, hist, first):
            """Batched emission for t-window tw from hist (ascending t slots)."""
            cbase = 0 if d == "f" else 2
            pse = ps_e.tile([K, 512], f32, tag="pse", name="pse")
            for k in range(2):
                nc.tensor.matmul(pse[:], lhsT=s_wout[:, cbase + k, :],
                                 rhs=hist[:, k, :, :], start=(k == 0), stop=(k == 1))
            dst = emit[:, tw * WIN:(tw + 1) * WIN, :].rearrange("k t b -> k (t b)")
            if first:
                nc.vector.tensor_scalar_add(dst[:], pse[:], s_bout[:, 0:1])
            else:
                nc.vector.tensor_tensor(dst[:], pse[:], dst[:], op=OP.add)

        # ---------- per-step pieces ----------
        def lstm_step(d, t, win, j):
            """One LSTM step for direction d at time t, window slot j."""
            psg = ps_g.tile([128, 8, BL], f32, tag="psg", name=f"psg{d}")
            # xw inject resets the bank; W_hh matmuls accumulate on top
            nc.tensor.matmul(psg[:].rearrange("p m b -> p (m b)"), lhsT=identb[:],
                             rhs=win[:, j, :, :].rearrange("p m b -> p (m b)"),
                             start=True, stop=False, skip_group_check=True)
            h = st_h[d]
            for m in range(8):
                for k in range(2):
                    nc.tensor.matmul(psg[:, m], lhsT=s_whh[d][:, k, m * 128:(m + 1) * 128],
                                     rhs=h[:, k, :], start=False, stop=(m == 7 and k == 1),
                                     skip_group_check=True)
            return psg

        def act_gates(d, psg):
            gates = tmp.tile([128, 8, BL], f32, tag=f"gates{d}", name=f"gates{d}")
            nc.scalar.activation(gates[:, 0:6], psg[:, 0:6], AF.Sigmoid)
            nc.scalar.activation(gates[:, 6:8], psg[:, 6:8], AF.Tanh)
            return gates

        def cell_mults(d, gates):
            ig = tmp.tile([128, 2, BL], f32, tag=f"ig{d}", name=f"ig{d}")
            nc.gpsimd.tensor_tensor(ig[:], gates[:, 0:2], gates[:, 6:8], op=OP.mult)
            fc = tmp.tile([128, 2, BL], f32, tag=f"fc{d}", name=f"fc{d}")
            nc.gpsimd.tensor_tensor(fc[:], gates[:, 2:4], st_c[d][:], op=OP.mult)
            return ig, fc

        def cell_update(d, ig, fc):
            # unmasked c update (frozen-region c is never read back)
            nc.vector.tensor_tensor(st_c[d][:], ig[:], fc[:], op=OP.add)

        def tanh_c(d):
            th = tmp.tile([128, 2, BL], f32, tag=f"th{d}", name=f"th{d}")
            nc.scalar.activation(th[:], st_c[d][:], AF.Tanh)
            return th

        def h_update(d, t, gates, th, hist, j):
            hh = tmp.tile([128, 2, BL], dt_w, tag=f"hh{d}", name=f"hh{d}")
            nc.vector.tensor_tensor(hh[:], gates[:, 4:6], th[:], op=OP.mult)
            m2 = mask_ap(t, 128, 2)
            nc.vector.copy_predicated(hist[:, :, j, :], m2, hh[:])
            nc.vector.copy_predicated(st_h[d][:], m2, hh[:])

        # ---------- prologue: build first windows ----------
        win_f, th_f = build_window_thunks("f", 0)
        for th in th_f:
            th()
        win_b, th_b = build_window_thunks("b", NW - 1)
        for th in th_b:
            th()
        cur_win["f"], cur_win["b"] = win_f, win_b
        hist_f = new_hist("f")
        nc.vector.memset(hist_f[:], 0.0)
        hist_b = new_hist("b")
        nc.vector.memset(hist_b[:], 0.0)
        cur_hist["f"], cur_hist["b"] = hist_f, hist_b
        prev_hist = {"f": None, "b": None}

        pending = []  # build thunks for next windows, drained ~2/iter

        # ---------- main interleaved loop ----------
        for i in range(T):
            blk, j = divmod(i, WIN)
            t_f = i
            t_b = T - 1 - i
            jb = WIN - 1 - j       # bwd hist slot (ascending t within window)

            if j == 0 and blk > 0:
                # windows blk-1 (fwd) and NW-blk (bwd t-window) just completed
                emit_window("f", blk - 1, prev_hist["f"], first=(blk - 1 <= 7))
                emit_window("b", NW - blk, prev_hist["b"], first=(NW - blk >= 8))

            # recurrence matmuls + activations, f then b staged
            psg_f = lstm_step("f", t_f, cur_win["f"], j)
            psg_b = lstm_step("b", t_b, cur_win["b"], jb)
            g_f = act_gates("f", psg_f)
            ig_f, fc_f = cell_mults("f", g_f)
            g_b = act_gates("b", psg_b)
            cell_update("f", ig_f, fc_f)
            ig_b, fc_b = cell_mults("b", g_b)
            th_ff = tanh_c("f")
            cell_update("b", ig_b, fc_b)
            th_bb = tanh_c("b")
            h_update("f", t_f, g_f, th_ff, cur_hist["f"], j)
            h_update("b", t_b, g_b, th_bb, cur_hist["b"], jb)

            # window prefetch bookkeeping
            if j == 0:
                pending = []
                if blk + 1 < NW:
                    w2, tks = build_window_thunks("f", blk + 1)
                    nw_f = w2
                    pending.extend(tks)
                    w2, tks = build_window_thunks("b", NW - 2 - blk)
                    nw_b = w2
                    pending.extend(tks)
                    h2_f = new_hist("f")
                    pending.append(lambda h=h2_f: nc.vector.memset(h[:], 0.0))
                    h2_b = new_hist("b")
                    pending.append(lambda h=h2_b: nc.vector.memset(h[:], 0.0))
            # drain ~3 build thunks per iteration (90 per block)
            for _ in range(3):
                if pending:
                    pending.pop(0)()
            if j == WIN - 1:
                while pending:
                    pending.pop(0)()
                prev_hist["f"], prev_hist["b"] = cur_hist["f"], cur_hist["b"]
                if blk + 1 < NW:
                    cur_win["f"], cur_win["b"] = nw_f, nw_b
                    cur_hist["f"], cur_hist["b"] = h2_f, h2_b

        # ---------- epilogue: last emissions + batched exp ----------
        emit_window("f", NW - 1, prev_hist["f"], first=False)
        emit_window("b", 0, prev_hist["b"], first=False)
        for tw in range(NW - 1, -1, -1):
            src = emit[:, tw * WIN:(tw + 1) * WIN, :].rearrange("k t b -> k (t b)")
            dst = expE[:, tw * WIN:(tw + 1) * WIN, :].rearrange("k t b -> k (t b)")
            nc.scalar.activation(dst[:], src[:], AF.Exp)

        # ---------- finalize thunks (interleaved into beta phase) ----------
        fin_thunks = []
        Uacc = fin.tile([K, BL], f32, tag="Uacc")
        nc.vector.memset(Uacc[:], 0.0)
        CH = 32
        TC = T // CH

        def unary_chunk(ci):
            t1 = fin.tile([K, TC * BL], f32, tag="t1")
            nc.vector.tensor_copy(t1[:], s_t1h[:, ci * TC:(ci + 1) * TC, :].rearrange("p t b -> p (t b)"))
            um = fin.tile([K, TC * BL], f32, tag="um")
            nc.vector.tensor_tensor(
                um[:], t1[:],
                emit[:, ci * TC:(ci + 1) * TC, :].rearrange("p t b -> p (t b)"),
                op=OP.mult)
            ur = fin.tile([K, BL], f32, tag="ur")
            umr = bass.AP(tensor=um.tensor, offset=um[:].offset,
                          ap=[um[:].ap[0], [1, BL], [BL, TC]])
            nc.vector.tensor_reduce(ur[:], umr, axis=mybir.AxisListType.X, op=OP.add)
            nc.vector.tensor_tensor(Uacc[:], Uacc[:], ur[:], op=OP.add)

        for ci in range(CH):
            fin_thunks.append(lambda ci=ci: unary_chunk(ci))

        TRbuf = fin.tile([128, NT128], f32, tag="TRbuf")

        def trans_chunk(i):
            tr = gat.tile([128, K], f32, tag="tr")
            nc.gpsimd.indirect_dma_start(
                out=tr[:], out_offset=None, in_=trans[:],
                in_offset=bass.IndirectOffsetOnAxis(ap=idxtag[:, i:i + 1], axis=0))
            nc.vector.tensor_tensor(tr[:], tr[:], s_tnx[:, i, :], op=OP.mult)
            nc.vector.tensor_reduce(TRbuf[:, i:i + 1], tr[:], axis=mybir.AxisListType.X, op=OP.add)

        for i in range(NT128):
            fin_thunks.append(lambda i=i: trans_chunk(i))

        # ---------- CRF beta recursion ----------
        for tstep in range(T - 2, -1, -1):
            tp1 = tstep + 1
            bp = tmp.tile([K, BL], f32, tag="bp", name="bp")
            nc.vector.tensor_tensor(bp[:], Bv[:], expE[:, tp1, :], op=OP.mult)
            psb = ps_s.tile([K, BL], f32, tag="pssm", name="psb")
            nc.tensor.matmul(psb[:], lhsT=s_expAT[:], rhs=bp[:], start=True, stop=True)
            nc.vector.copy_predicated(Bv[:], mask_ap(tp1, K, 1), psb[:])

            if tstep % RESCALE == 0 and tstep > 0:
                ri = tstep // RESCALE
                pss = ps_s.tile([1, BL], f32, tag="pssm", name="pss")
                nc.tensor.matmul(pss[:], lhsT=ones[0:K, 0:1], rhs=Bv[:], start=True, stop=True)
                nc.vector.copy_predicated(sums[:, ri, :], mask_ap(tstep, 1, 1), pss[:])
                rr = tmp.tile([1, BL], f32, tag="rr")
                nc.vector.reciprocal(rr[:], pss[:])
                psr = ps_s.tile([K, BL], f32, tag="pssm", name="psr")
                nc.tensor.matmul(psr[:], lhsT=ones[0:1, 0:K], rhs=rr[:], start=True, stop=True)
                # fold the rescale into the expE slice the next step consumes
                nc.vector.tensor_tensor(expE[:, tstep, :], expE[:, tstep, :], psr[:], op=OP.mult)

            if fin_thunks and tstep % 5 == 0:
                fin_thunks.pop(0)()

        while fin_thunks:
            fin_thunks.pop(0)()

        # ---------- final assembly ----------
        zt = fin.tile([K, BL], f32, tag="zt")
        nc.vector.tensor_tensor(zt[:], Bv[:], expE[:, 0, :], op=OP.mult)
        psz = ps_s.tile([1, BL], f32, tag="pssm")
        nc.tensor.matmul(psz[:], lhsT=ones[0:K, 0:1], rhs=zt[:], start=True, stop=True)
        logZ = fin.tile([1, BL], f32, tag="logZ")
        nc.scalar.activation(logZ[:], psz[:], AF.Ln)

        # deferred ln of the rescale sums: one batched Ln + strided reduce
        lns = fin.tile([1, NRS, BL], f32, tag="lns")
        nc.scalar.activation(lns[:].rearrange("p r b -> p (r b)"),
                             sums[:].rearrange("p r b -> p (r b)"), AF.Ln)
        lsum = fin.tile([1, BL], f32, tag="lsum")
        lns_ap = bass.AP(tensor=lns.tensor, offset=lns[:].offset,
                         ap=[lns[:].ap[0], [1, BL], [BL, NRS]])
        nc.vector.tensor_reduce(lsum[:], lns_ap, axis=mybir.AxisListType.X, op=OP.add)
        nc.vector.tensor_tensor(logZ[:], logZ[:], lsum[:], op=OP.add)

        # unary total
        psu = ps_s.tile([1, BL], f32, tag="pssm")
        nc.tensor.matmul(psu[:], lhsT=ones[0:K, 0:1], rhs=Uacc[:], start=True, stop=True)
        score = fin.tile([1, BL], f32, tag="score")
        nc.vector.tensor_copy(score[:], psu[:])

        # transition total: colsum TRbuf then per-b strided reduce
        QT = T // 128
        pstr = ps_s.tile([1, NT128], f32, tag="pssm")
        nc.tensor.matmul(pstr[:], lhsT=ones[:, 0:1], rhs=TRbuf[:], start=True, stop=True)
        trv = fin.tile([1, BL], f32, tag="trv")
        ptr_ap = bass.AP(tensor=pstr.tensor, offset=pstr[:].offset,
                         ap=[pstr[:].ap[0], [QT, BL], [1, QT]])
        nc.vector.tensor_reduce(trv[:], ptr_ap, axis=mybir.AxisListType.X, op=OP.add)

        # loss = logZ - (score + trans)
        nc.vector.tensor_tensor(score[:], score[:], trv[:], op=OP.add)
        res = fin.tile([1, BL], f32, tag="res")
        nc.vector.tensor_tensor(res[:], logZ[:], score[:], op=OP.subtract)
        nc.sync.dma_start(out=out_loss[:], in_=res[:])

    nc.compile()
    return nc, names


# torch gate order (i, f, g, o) -> kernel order (i, f, o, g)
def _perm_rows(w):
    return np.concatenate([w[0:2 * H], w[3 * H:4 * H], w[2 * H:3 * H]], axis=0)


def _prep_core(inputs, k, dt_np):
    """Build the per-core input map (host-side index plumbing only)."""
    s = slice(k * BL, (k + 1) * BL)
    sent = np.asarray(inputs["sentences"][s])          # (16, 512) i32
    tags = np.asarray(inputs["tags"][s])               # (16, 512) i32
    mask = (sent != PAD_IDX)
    # toks in (w, j, b) order so gathered/psw columns are (j, b)
    toks = sent.reshape(BL, NW, WIN).transpose(1, 2, 0).reshape(T * BL, 1)
    oh = (tags[:, :, None] == np.arange(K)[None, None, :])
    tags1h = (oh & mask[:, :, None]).transpose(2, 1, 0).reshape(K, T * BL)
    tnx = np.zeros((BL, T, K), np.float32)
    tnx[:, :-1, :] = (oh[:, 1:, :] & mask[:, 1:, None]).astype(np.float32)
    m = {
        "toks": toks.astype(np.int32),
        "masku": mask.T.astype(np.uint8).reshape(1, T * BL),
        "tags1h": tags1h.astype(np.uint8),
        "tagsnx": tnx.reshape(T * BL, K).astype(np.float32),
        "tagsfl": tags.reshape(T * BL, 1).astype(np.int32),
        "emb": np.asarray(inputs["embedding"]).astype(dt_np),
        "wih_f": np.ascontiguousarray(_perm_rows(np.asarray(inputs["w_ih_f"])).T).astype(dt_np),
        "wih_b": np.ascontiguousarray(_perm_rows(np.asarray(inputs["w_ih_b"])).T).astype(dt_np),
        "whh_f": np.ascontiguousarray(_perm_rows(np.asarray(inputs["w_hh_f"])).T).astype(dt_np),
        "whh_b": np.ascontiguousarray(_perm_rows(np.asarray(inputs["w_hh_b"])).T).astype(dt_np),
        "bih_f": np.ascontiguousarray(_perm_rows(np.asarray(inputs["b_f"])).reshape(8, 128).T).astype(np.float32),
        "bih_b": np.ascontiguousarray(_perm_rows(np.asarray(inputs["b_b"])).reshape(8, 128).T).astype(np.float32),
        "woutT": np.ascontiguousarray(np.asarray(inputs["w_out"]).T.reshape(4, 128, K)).astype(dt_np),
        "bout": np.asarray(inputs["b_out"]).reshape(K, 1).astype(np.float32),
        "expAT": np.ascontiguousarray(np.exp(np.asarray(inputs["transition"], np.float64)).T).astype(np.float32),
        "trans": np.asarray(inputs["transition"], np.float32),
    }
    return m


def kernel(**inputs):
    import ml_dtypes
    from concourse import mybir
    from concourse.bass_utils import run_bass_kernel_spmd

    use_bf16 = _cache.get("use_bf16", True)
    key = ("prog", use_bf16)
    if key not in _cache:
        dt_w = mybir.dt.bfloat16 if use_bf16 else mybir.dt.float32
        _cache[key] = _build_program(dt_w)
    nc, names = _cache[key]
    dt_np = ml_dtypes.bfloat16 if use_bf16 else np.float32

    in_maps = []
    for k in range(NCORES):
        m = _prep_core(inputs, k, dt_np)
        in_maps.append({names[kk]: vv for kk, vv in m.items()})

    res = run_bass_kernel_spmd(nc, in_maps, core_ids=list(range(NCORES)),
                               **_cache.get("run_kwargs", {}))
    out = np.concatenate([r[names["out"]].reshape(BL) for r in res.results])
    _cache["last_results"] = res
    return out.astype(np.float32)


# revision 15
# speedup vs baseline: 2.4144x; 1.0381x over previous
"""BiLSTM-CRF loss kernel for Trainium2 (8 NeuronCores, data-parallel over batch).

v2 design (per core, B_loc=16 sequences):
  - Forward and backward LSTM directions run INTERLEAVED in a single
    512-iteration loop (iter i: fwd step t=i, bwd step t=511-i) so the two
    independent recurrence chains fill each other's engine stalls.
  - Gate order host-permuted to (i, f, o, g) so activations are 2 instrs
    per step: sigmoid over 96 cols + tanh over 32 cols.
  - xw window injected into the gate PSUM via an identity matmul
    (start=True) before the 16 W_hh matmuls accumulate on top; the ACT
    engine reads gates straight from PSUM (no separate gate-add).
  - c update is unmasked (pad mask is a suffix per sequence; the unfrozen
    c is never read back and stays bounded), h masked via copy_predicated.
  - h history per 32-step window -> batched emission matmuls (4/window/dir)
    instead of 2 per step.
  - All exp() for the CRF batched per window in the epilogue: keeps the
    sigmoid+tanh activation tables resident all of phase 1 (no
    ACT_TABLE_LOAD thrash).
  - CRF log-partition via backward beta recursion in exp space as a
    separate 511-step phase; rescaling folded into the next step's expE
    slice (off the critical path), ln() of the scales deferred to one
    batched instruction at the end.
  - Gold-path score (unary + transition gather) interleaved into the beta
    phase.
"""

import numpy as np

PAD_IDX = 0
VOCAB, K, E, H = 30000, 20, 256, 256
B, T = 128, 512
NCORES = 8
BL = B // NCORES          # 16 sequences per core
WIN = 32                  # proj window (time steps)
NW = T // WIN             # 16 windows
RESCALE = 8               # CRF rescale interval

_cache = {}


def _build_program(dt_w):
    """Build the SPMD Bass program. dt_w: matmul weight/stream dtype."""
    from contextlib import ExitStack
    import concourse.bass as bass
    import concourse.bacc as bacc
    import concourse.tile as tile
    from concourse import mybir
    from concourse.masks import make_identity

    f32 = mybir.dt.float32
    i32 = mybir.dt.int32

    nc = bacc.Bacc(None, target_bir_lowering=False, debug=False)
    names = {}

    with ExitStack() as ctx:
        tc = ctx.enter_context(tile.TileContext(nc))
        dram = ctx.enter_context(tc.tile_pool(name="dram", bufs=1, space="DRAM"))

        def din(key, shape, dt=f32):
            t = dram.tile(shape, dt, kind="ExternalInput", name=key)
            names[key] = t.tensor.name
            return t

        emb = din("emb", [VOCAB, E], dt_w)
        toks = din("toks", [T * BL, 1], i32)          # (w, j, b) window/j-major
        masku = din("masku", [1, T * BL], mybir.dt.uint8)  # col = t*16+b
        tags1h = din("tags1h", [K, T * BL], mybir.dt.uint8)  # one-hot(tag) * mask
        tagsnx = din("tagsnx", [T * BL, K])           # shifted one-hot * mask, f32
        tagsfl = din("tagsfl", [T * BL, 1], i32)      # tag ids, b-major
        wih = {d: din(f"wih_{d}", [E, 4 * H], dt_w) for d in "fb"}
        whh = {d: din(f"whh_{d}", [E, 4 * H], dt_w) for d in "fb"}
        bih = {d: din(f"bih_{d}", [128, 8]) for d in "fb"}
        woutT = din("woutT", [4, 128, K], dt_w)       # chunks: Fk0,Fk1,Bk0,Bk1
        bout = din("bout", [K, 1])
        expAT = din("expAT", [K, K])                  # exp(transition).T
        trans = din("trans", [K, K])                  # raw, for row gather
        out_loss = dram.tile([1, BL], f32, kind="ExternalOutput")
        names["out"] = out_loss.tensor.name

        sg = ctx.enter_context(tc.tile_pool(name="sg", bufs=1))       # singles
        tmp = ctx.enter_context(tc.tile_pool(name="tmp", bufs=3))     # step temps
        gat = ctx.enter_context(tc.tile_pool(name="gat", bufs=4))     # gather tiles
        winp = ctx.enter_context(tc.tile_pool(name="winp", bufs=2))   # xw windows
        hhp = ctx.enter_context(tc.tile_pool(name="hhp", bufs=2))     # h history
        xtw = ctx.enter_context(tc.tile_pool(name="xtw", bufs=2))
        fin = ctx.enter_context(tc.tile_pool(name="fin", bufs=3))     # finalize
        ps_g = ctx.enter_context(tc.tile_pool(name="ps_g", bufs=2, space="PSUM"))
        ps_w = ctx.enter_context(tc.tile_pool(name="ps_w", bufs=1, space="PSUM"))
        ps_e = ctx.enter_context(tc.tile_pool(name="ps_e", bufs=1, space="PSUM"))
        ps_s = ctx.enter_context(tc.tile_pool(name="ps_s", bufs=2, space="PSUM"))

        # ---- resident SBUF tensors ----
        s_wih = {d: sg.tile([128, 2, 4 * H], dt_w, tag=f"wih{d}", name=f"wih{d}") for d in "fb"}
        s_whh = {d: sg.tile([128, 2, 4 * H], dt_w, tag=f"whh{d}", name=f"whh{d}") for d in "fb"}
        for d in "fb":
            nc.sync.dma_start(out=s_wih[d][:], in_=wih[d][:].rearrange("(k p) m -> p k m", p=128))
            nc.sync.dma_start(out=s_whh[d][:], in_=whh[d][:].rearrange("(k p) m -> p k m", p=128))
        s_bih = {d: sg.tile([128, 8], f32, tag=f"bih{d}", name=f"bih{d}") for d in "fb"}
        for d in "fb":
            nc.sync.dma_start(out=s_bih[d][:], in_=bih[d][:])
        s_wout = sg.tile([128, 4, K], dt_w, tag="wout")
        nc.sync.dma_start(out=s_wout[:], in_=woutT[:].rearrange("c p k -> p c k"))
        s_bout = sg.tile([K, 1], f32, tag="bout")
        nc.sync.dma_start(out=s_bout[:], in_=bout[:])
        s_expAT = sg.tile([K, K], f32, tag="expAT")
        nc.sync.dma_start(out=s_expAT[:], in_=expAT[:])

        ones = sg.tile([128, K], f32, tag="ones")
        nc.vector.memset(ones[:], 1.0)
        identb = sg.tile([128, 128], dt_w, tag="identb")
        make_identity(nc, identb[:])

        # mask replica: (128, T, BL), col = t*16+b, broadcast across partitions
        maskrep = sg.tile([128, T, BL], mybir.dt.uint8, tag="maskrep")
        nc.sync.dma_start(
            out=maskrep[:],
            in_=bass.AP(tensor=masku.tensor, offset=masku[:].offset,
                        ap=[[0, 128], [BL, T], [1, BL]]),
        )

        emit = sg.tile([K, T, BL], f32, tag="emit")
        expE = sg.tile([K, T, BL], f32, tag="expE")

        # gather indices resident (one upfront DMA each)
        NT128 = T * BL // 128
        idxall = sg.tile([128, NT128], i32, tag="idxall")
        nc.sync.dma_start(out=idxall[:],
                          in_=bass.AP(tensor=toks.tensor, offset=toks[:].offset,
                                      ap=[[1, 128], [128, NT128]]))
        idxtag = sg.tile([128, NT128], i32, tag="idxtag")
        nc.sync.dma_start(out=idxtag[:],
                          in_=bass.AP(tensor=tagsfl.tensor, offset=tagsfl[:].offset,
                                      ap=[[1, 128], [128, NT128]]))
        s_t1h = sg.tile([K, T, BL], mybir.dt.uint8, tag="s_t1h")
        nc.sync.dma_start(out=s_t1h[:].rearrange("k t b -> k (t b)"), in_=tags1h[:])
        s_tnx = sg.tile([128, NT128, K], f32, tag="s_tnx")
        nc.sync.dma_start(out=s_tnx[:],
                          in_=tagsnx[:].rearrange("(n p) k -> p n k", p=128))

        # LSTM states (h in dt_w for matmul rhs, c in f32)
        st_h = {d: sg.tile([128, 2, BL], dt_w, tag=f"h{d}", name=f"h{d}") for d in "fb"}
        st_c = {d: sg.tile([128, 2, BL], f32, tag=f"c{d}", name=f"c{d}") for d in "fb"}
        for d in "fb":
            nc.vector.memset(st_h[d][:], 0.0)
            nc.vector.memset(st_c[d][:], 0.0)

        # CRF beta state (exp space) + deferred-ln scale buffer
        Bv = sg.tile([K, BL], f32, tag="Bv")
        nc.vector.memset(Bv[:], 1.0)
        NRS = T // RESCALE
        sums = sg.tile([1, NRS, BL], f32, tag="sums")
        nc.vector.memset(sums[:], 1.0)

        AF = mybir.ActivationFunctionType
        OP = mybir.AluOpType

        def mask_ap(t, parts, reps):
            """maskrep[:parts, t, :] replicated reps times along a middle dim."""
            base = maskrep[0:parts, t, :]
            if reps == 1:
                return base
            return bass.AP(tensor=base.tensor, offset=base.offset,
                           ap=[base.ap[0], [0, reps], [1, BL]])

        # warm-up matmuls: make PE's clock pass every weight-producing op so
        # steady-state matmuls carry at most one semaphore wait
        for wt in [s_wih["f"][:, 0, 0:1], s_wih["b"][:, 0, 0:1],
                   s_whh["f"][:, 0, 0:1], s_whh["b"][:, 0, 0:1],
                   s_wout[:, 0, 0:1], identb[:, 0:1]]:
            psd = ps_s.tile([1, 1], f32, tag="pssm")
            nc.tensor.matmul(psd[:], lhsT=wt, rhs=wt, start=True, stop=True)
        psd = ps_s.tile([1, 1], f32, tag="pssm")
        nc.tensor.matmul(psd[:], lhsT=s_expAT[0:K, 0:1], rhs=s_expAT[0:K, 0:1], start=True, stop=True)
        psd = ps_s.tile([1, 1], f32, tag="pssm")
        nc.tensor.matmul(psd[:], lhsT=ones[0:1, 0:1], rhs=ones[0:1, 0:1], start=True, stop=True)

        # ---------- window machinery ----------
        # win layout: (128, WIN, 8, BL) -> inject rhs win[:, j, :, :] is one
        # contiguous 128-col slice.  h_hist: (128, 2, WIN, BL) (k, j, b).
        cur_win = {}
        cur_hist = {}

        def build_window_thunks(d, tw):
            """Return (win_tile, thunk list) building xw window for t-window tw."""
            win = winp.tile([128, WIN, 8, BL], dt_w, tag=f"win{d}", name=f"win{d}")
            xT = xtw.tile([128, 2, 512], dt_w, tag=f"xT{d}", name=f"xT{d}")
            thunks = []
            pst_box = {}

            def gather(g):
                xg = gat.tile([128, E], dt_w, tag=f"xg{d}", name=f"xg{d}")
                nc.gpsimd.indirect_dma_start(
                    out=xg[:], out_offset=None, in_=emb[:],
                    in_offset=bass.IndirectOffsetOnAxis(ap=idxall[:, tw * 4 + g:tw * 4 + g + 1], axis=0),
                )
                pst_box[g] = xg

            def tp(g, k):
                xg = pst_box[g]
                pst = ps_s.tile([128, 128], dt_w, tag="pssm", name="pst")
                nc.tensor.transpose(out=pst[:], in_=xg[:, k * 128:(k + 1) * 128], identity=identb[:])
                nc.vector.tensor_copy(xT[:, k, g * 128:(g + 1) * 128], pst[:])

            for g in range(4):
                thunks.append(lambda g=g: gather(g))
                for k in range(2):
                    thunks.append(lambda g=g, k=k: tp(g, k))

            def proj(m):
                psw = ps_w.tile([128, 512], f32, tag="psw", name="psw")
                for k in range(2):
                    nc.tensor.matmul(psw[:], lhsT=s_wih[d][:, k, m * 128:(m + 1) * 128],
                                     rhs=xT[:, k, :], start=(k == 0), stop=(k == 1))
                # psw cols are (j, b); win[:, :, m, :] has free dims (j, b)
                nc.vector.tensor_scalar_add(win[:, :, m, :], psw[:], s_bih[d][:, m:m + 1])

            for m in range(8):
                thunks.append(lambda m=m: proj(m))
            return win, thunks

        def new_hist(d):
            hist = hhp.tile([128, 2, WIN, BL], dt_w, tag=f"hist{d}", name=f"hist{d}")
            return hist

        def emit_window(d, tw, hist, first):
            """Batched emission for t-window tw from hist (ascending t slots)."""
            cbase = 0 if d == "f" else 2
            pse = ps_e.tile([K, 512], f32, tag="pse", name="pse")
            for k in range(2):
                nc.tensor.matmul(pse[:], lhsT=s_wout[:, cbase + k, :],
                                 rhs=hist[:, k, :, :], start=(k == 0), stop=(k == 1))
            dst = emit[:, tw * WIN:(tw + 1) * WIN, :].rearrange("k t b -> k (t b)")
            if first:
                nc.vector.tensor_scalar_add(dst[:], pse[:], s_bout[:, 0:1])
            else:
                nc.vector.tensor_tensor(dst[:], pse[:], dst[:], op=OP.add)

        # ---------- per-step pieces ----------
        # gate psum split (i,f,o) vs (g): the g matmuls are emitted first so
        # tanh(g) runs during the (i,f,o) matmul burst.
        def lstm_step(d, t, win, j):
            """One LSTM step for direction d at time t, window slot j."""
            psgg = ps_g.tile([128, 2, BL], f32, tag="psgg", name=f"psgg{d}")
            psgi = ps_g.tile([128, 6, BL], f32, tag="psgi", name=f"psgi{d}")
            h = st_h[d]
            nc.tensor.matmul(psgg[:].rearrange("p m b -> p (m b)"), lhsT=identb[:],
                             rhs=win[:, j, 6:8, :].rearrange("p m b -> p (m b)"),
                             start=True, stop=False, skip_group_check=True)
            for m in range(6, 8):
                for k in range(2):
                    nc.tensor.matmul(psgg[:, m - 6], lhsT=s_whh[d][:, k, m * 128:(m + 1) * 128],
                                     rhs=h[:, k, :], start=False, stop=(m == 7 and k == 1),
                                     skip_group_check=True)
            nc.tensor.matmul(psgi[:].rearrange("p m b -> p (m b)"), lhsT=identb[:],
                             rhs=win[:, j, 0:6, :].rearrange("p m b -> p (m b)"),
                             start=True, stop=False, skip_group_check=True)
            for m in range(6):
                for k in range(2):
                    nc.tensor.matmul(psgi[:, m], lhsT=s_whh[d][:, k, m * 128:(m + 1) * 128],
                                     rhs=h[:, k, :], start=False, stop=(m == 5 and k == 1),
                                     skip_group_check=True)
            return psgi, psgg

        def act_tanh_g(d, psgg):
            gg = tmp.tile([128, 2, BL], f32, tag=f"gg{d}", name=f"gg{d}")
            nc.scalar.activation(gg[:], psgg[:], AF.Tanh)
            return gg

        def act_sig(d, psgi):
            gates = tmp.tile([128, 6, BL], f32, tag=f"gates{d}", name=f"gates{d}")
            nc.scalar.activation(gates[:], psgi[:], AF.Sigmoid)
            return gates

        def cell_mults(d, gates, gg):
            # ig on gpsimd, fc on vector: the two products run on parallel engines
            ig = tmp.tile([128, 2, BL], f32, tag=f"ig{d}", name=f"ig{d}")
            nc.gpsimd.tensor_tensor(ig[:], gates[:, 0:2], gg[:], op=OP.mult)
            fc = tmp.tile([128, 2, BL], f32, tag=f"fc{d}", name=f"fc{d}")
            nc.vector.tensor_tensor(fc[:], gates[:, 2:4], st_c[d][:], op=OP.mult)
            return ig, fc

        def cell_update(d, ig, fc):
            # unmasked c update (frozen-region c is never read back)
            nc.vector.tensor_tensor(st_c[d][:], ig[:], fc[:], op=OP.add)

        def tanh_c(d):
            th = tmp.tile([128, 2, BL], f32, tag=f"th{d}", name=f"th{d}")
            nc.scalar.activation(th[:], st_c[d][:], AF.Tanh)
            return th

        def h_mult(d, gates, th):
            hh = tmp.tile([128, 2, BL], dt_w, tag=f"hh{d}", name=f"hh{d}")
            nc.vector.tensor_tensor(hh[:], gates[:, 4:6], th[:], op=OP.mult)
            return hh

        def h_state(d, t, hh):
            nc.vector.copy_predicated(st_h[d][:], mask_ap(t, 128, 2), hh[:])

        def h_hist_write(d, t, hh, hist, j):
            nc.vector.copy_predicated(hist[:, :, j, :], mask_ap(t, 128, 2), hh[:])

        # ---------- prologue: build first windows ----------
        win_f, th_f = build_window_thunks("f", 0)
        for th in th_f:
            th()
        win_b, th_b = build_window_thunks("b", NW - 1)
        for th in th_b:
            th()
        cur_win["f"], cur_win["b"] = win_f, win_b
        hist_f = new_hist("f")
        nc.vector.memset(hist_f[:], 0.0)
        hist_b = new_hist("b")
        nc.vector.memset(hist_b[:], 0.0)
        cur_hist["f"], cur_hist["b"] = hist_f, hist_b
        prev_hist = {"f": None, "b": None}

        pending = []  # build thunks for next windows, drained ~2/iter

        # ---------- main interleaved loop ----------
        for i in range(T):
            blk, j = divmod(i, WIN)
            t_f = i
            t_b = T - 1 - i
            jb = WIN - 1 - j       # bwd hist slot (ascending t within window)

            if j == 0 and blk > 0:
                # windows blk-1 (fwd) and NW-blk (bwd t-window) just completed
                emit_window("f", blk - 1, prev_hist["f"], first=(blk - 1 <= 7))
                emit_window("b", NW - blk, prev_hist["b"], first=(NW - blk >= 8))

            # recurrence matmuls + activations, f then b staged
            psgi_f, psgg_f = lstm_step("f", t_f, cur_win["f"], j)
            psgi_b, psgg_b = lstm_step("b", t_b, cur_win["b"], jb)
            gg_f = act_tanh_g("f", psgg_f)
            g_f = act_sig("f", psgi_f)
            gg_b = act_tanh_g("b", psgg_b)
            ig_f, fc_f = cell_mults("f", g_f, gg_f)
            g_b = act_sig("b", psgi_b)
            cell_update("f", ig_f, fc_f)
            ig_b, fc_b = cell_mults("b", g_b, gg_b)
            th_ff = tanh_c("f")
            cell_update("b", ig_b, fc_b)
            th_bb = tanh_c("b")
            hh_f = h_mult("f", g_f, th_ff)
            h_state("f", t_f, hh_f)
            hh_b = h_mult("b", g_b, th_bb)
            h_state("b", t_b, hh_b)
            h_hist_write("f", t_f, hh_f, cur_hist["f"], j)
            h_hist_write("b", t_b, hh_b, cur_hist["b"], jb)

            # window prefetch bookkeeping
            if j == 0:
                pending = []
                if blk + 1 < NW:
                    w2, tks = build_window_thunks("f", blk + 1)
                    nw_f = w2
                    pending.extend(tks)
                    w2, tks = build_window_thunks("b", NW - 2 - blk)
                    nw_b = w2
                    pending.extend(tks)
                    h2_f = new_hist("f")
                    pending.append(lambda h=h2_f: nc.vector.memset(h[:], 0.0))
                    h2_b = new_hist("b")
                    pending.append(lambda h=h2_b: nc.vector.memset(h[:], 0.0))
            # drain ~3 build thunks per iteration (90 per block)
            for _ in range(3):
                if pending:
                    pending.pop(0)()
            if j == WIN - 1:
                while pending:
                    pending.pop(0)()
                prev_hist["f"], prev_hist["b"] = cur_hist["f"], cur_hist["b"]
                if blk + 1 < NW:
                    cur_win["f"], cur_win["b"] = nw_f, nw_b
                    cur_hist["f"], cur_hist["b"] = h2_f, h2_b

        # ---------- epilogue: last emissions + batched exp ----------
        emit_window("f", NW - 1, prev_hist["f"], first=False)
        emit_window("b", 0, prev_hist["b"], first=False)
        for tw in range(NW - 1, -1, -1):
            src = emit[:, tw * WIN:(tw + 1) * WIN, :].rearrange("k t b -> k (t b)")
            dst = expE[:, tw * WIN:(tw + 1) * WIN, :].rearrange("k t b -> k (t b)")
            nc.scalar.activation(dst[:], src[:], AF.Exp)

        # ---------- finalize thunks (interleaved into beta phase) ----------
        fin_thunks = []
        Uacc = fin.tile([K, BL], f32, tag="Uacc")
        nc.vector.memset(Uacc[:], 0.0)
        CH = 32
        TC = T // CH

        def unary_chunk(ci):
            t1 = fin.tile([K, TC * BL], f32, tag="t1")
            nc.vector.tensor_copy(t1[:], s_t1h[:, ci * TC:(ci + 1) * TC, :].rearrange("p t b -> p (t b)"))
            um = fin.tile([K, TC * BL], f32, tag="um")
            nc.vector.tensor_tensor(
                um[:], t1[:],
                emit[:, ci * TC:(ci + 1) * TC, :].rearrange("p t b -> p (t b)"),
                op=OP.mult)
            ur = fin.tile([K, BL], f32, tag="ur")
            umr = bass.AP(tensor=um.tensor, offset=um[:].offset,
                          ap=[um[:].ap[0], [1, BL], [BL, TC]])
            nc.vector.tensor_reduce(ur[:], umr, axis=mybir.AxisListType.X, op=OP.add)
            nc.vector.tensor_tensor(Uacc[:], Uacc[:], ur[:], op=OP.add)

        for ci in range(CH):
            fin_thunks.append(lambda ci=ci: unary_chunk(ci))

        TRbuf = fin.tile([128, NT128], f32, tag="TRbuf")

        def trans_chunk(i):
            tr = gat.tile([128, K], f32, tag="tr")
            nc.gpsimd.indirect_dma_start(
                out=tr[:], out_offset=None, in_=trans[:],
                in_offset=bass.IndirectOffsetOnAxis(ap=idxtag[:, i:i + 1], axis=0))
            nc.vector.tensor_tensor(tr[:], tr[:], s_tnx[:, i, :], op=OP.mult)
            nc.vector.tensor_reduce(TRbuf[:, i:i + 1], tr[:], axis=mybir.AxisListType.X, op=OP.add)

        for i in range(NT128):
            fin_thunks.append(lambda i=i: trans_chunk(i))

        # ---------- CRF beta recursion (2 independent batch strands) ----------
        NS = 2
        SB = BL // NS

        def scol(s):
            return slice(s * SB, (s + 1) * SB)

        def mask_sap(t, parts, s):
            base = maskrep[0:parts, t, scol(s)]
            return base

        for tstep in range(T - 2, -1, -1):
            tp1 = tstep + 1
            bps = []
            for s in range(NS):
                bp = tmp.tile([K, SB], f32, tag=f"bp{s}", name=f"bp{s}")
                nc.vector.tensor_tensor(bp[:], Bv[:, scol(s)], expE[:, tp1, scol(s)], op=OP.mult)
                bps.append(bp)
            psbs = []
            for s in range(NS):
                psb = ps_s.tile([K, SB], f32, tag="pssm", name=f"psb{s}")
                nc.tensor.matmul(psb[:], lhsT=s_expAT[:], rhs=bps[s][:], start=True, stop=True)
                psbs.append(psb)
            for s in range(NS):
                nc.vector.copy_predicated(Bv[:, scol(s)], mask_sap(tp1, K, s), psbs[s][:])

            if tstep % RESCALE == 0 and tstep > 0:
                ri = tstep // RESCALE
                for s in range(NS):
                    pss = ps_s.tile([1, SB], f32, tag="pssm", name=f"pss{s}")
                    nc.tensor.matmul(pss[:], lhsT=ones[0:K, 0:1], rhs=Bv[:, scol(s)], start=True, stop=True)
                    nc.vector.copy_predicated(sums[:, ri, scol(s)], mask_sap(tstep, 1, s), pss[:])
                    rr = tmp.tile([1, SB], f32, tag=f"rr{s}")
                    nc.vector.reciprocal(rr[:], pss[:])
                    psr = ps_s.tile([K, SB], f32, tag="pssm", name=f"psr{s}")
                    nc.tensor.matmul(psr[:], lhsT=ones[0:1, 0:K], rhs=rr[:], start=True, stop=True)
                    # fold the rescale into the expE slice the next step consumes
                    nc.vector.tensor_tensor(expE[:, tstep, scol(s)], expE[:, tstep, scol(s)],
                                            psr[:], op=OP.mult)

            if fin_thunks and tstep % 5 == 0:
                fin_thunks.pop(0)()

        while fin_thunks:
            fin_thunks.pop(0)()

        # ---------- final assembly ----------
        zt = fin.tile([K, BL], f32, tag="zt")
        nc.vector.tensor_tensor(zt[:], Bv[:], expE[:, 0, :], op=OP.mult)
        psz = ps_s.tile([1, BL], f32, tag="pssm")
        nc.tensor.matmul(psz[:], lhsT=ones[0:K, 0:1], rhs=zt[:], start=True, stop=True)
        logZ = fin.tile([1, BL], f32, tag="logZ")
        nc.scalar.activation(logZ[:], psz[:], AF.Ln)

        # deferred ln of the rescale sums: one batched Ln + strided reduce
        lns = fin.tile([1, NRS, BL], f32, tag="lns")
        nc.scalar.activation(lns[:].rearrange("p r b -> p (r b)"),
                             sums[:].rearrange("p r b -> p (r b)"), AF.Ln)
        lsum = fin.tile([1, BL], f32, tag="lsum")
        lns_ap = bass.AP(tensor=lns.tensor, offset=lns[:].offset,
                         ap=[lns[:].ap[0], [1, BL], [BL, NRS]])
        nc.vector.tensor_reduce(lsum[:], lns_ap, axis=mybir.AxisListType.X, op=OP.add)
        nc.vector.tensor_tensor(logZ[:], logZ[:], lsum[:], op=OP.add)

        # unary total
        psu = ps_s.tile([1, BL], f32, tag="pssm")
        nc.tensor.matmul(psu[:], lhsT=ones[0:K, 0:1], rhs=Uacc[:], start=True, stop=True)
        score = fin.tile([1, BL], f32, tag="score")
        nc.vector.tensor_copy(score[:], psu[:])

        # transition total: colsum TRbuf then per-b strided reduce
        QT = T // 128
        pstr = ps_s.tile([1, NT128], f32, tag="pssm")
        nc.tensor.matmul(pstr[:], lhsT=ones[:, 0:1], rhs=TRbuf[:], start=True, stop=True)
        trv = fin.tile([1, BL], f32, tag="trv")
        ptr_ap = bass.AP(tensor=pstr.tensor, offset=pstr[:].offset,
                         ap=[pstr[:].ap[0], [QT, BL], [1, QT]])
        nc.vector.tensor_reduce(trv[:], ptr_ap, axis=mybir.AxisListType.X, op=OP.add)

        # loss = logZ - (score + trans)
        nc.vector.tensor_tensor(score[:], score[:], trv[:], op=OP.add)
        res = fin.tile([1, BL], f32, tag="res")
        nc.vector.tensor_tensor(res[:], logZ[:], score[:], op=OP.subtract)
        nc.sync.dma_start(out=out_loss[:], in_=res[:])

    nc.compile()
    return nc, names


# torch gate order (i, f, g, o) -> kernel order (i, f, o, g)
def _perm_rows(w):
    return np.concatenate([w[0:2 * H], w[3 * H:4 * H], w[2 * H:3 * H]], axis=0)


def _prep_core(inputs, k, dt_np):
    """Build the per-core input map (host-side index plumbing only)."""
    s = slice(k * BL, (k + 1) * BL)
    sent = np.asarray(inputs["sentences"][s])          # (16, 512) i32
    tags = np.asarray(inputs["tags"][s])               # (16, 512) i32
    mask = (sent != PAD_IDX)
    # toks in (w, j, b) order so gathered/psw columns are (j, b)
    toks = sent.reshape(BL, NW, WIN).transpose(1, 2, 0).reshape(T * BL, 1)
    oh = (tags[:, :, None] == np.arange(K)[None, None, :])
    tags1h = (oh & mask[:, :, None]).transpose(2, 1, 0).reshape(K, T * BL)
    tnx = np.zeros((BL, T, K), np.float32)
    tnx[:, :-1, :] = (oh[:, 1:, :] & mask[:, 1:, None]).astype(np.float32)
    m = {
        "toks": toks.astype(np.int32),
        "masku": mask.T.astype(np.uint8).reshape(1, T * BL),
        "tags1h": tags1h.astype(np.uint8),
        "tagsnx": tnx.reshape(T * BL, K).astype(np.float32),
        "tagsfl": tags.reshape(T * BL, 1).astype(np.int32),
        "emb": np.asarray(inputs["embedding"]).astype(dt_np),
        "wih_f": np.ascontiguousarray(_perm_rows(np.asarray(inputs["w_ih_f"])).T).astype(dt_np),
        "wih_b": np.ascontiguousarray(_perm_rows(np.asarray(inputs["w_ih_b"])).T).astype(dt_np),
        "whh_f": np.ascontiguousarray(_perm_rows(np.asarray(inputs["w_hh_f"])).T).astype(dt_np),
        "whh_b": np.ascontiguousarray(_perm_rows(np.asarray(inputs["w_hh_b"])).T).astype(dt_np),
        "bih_f": np.ascontiguousarray(_perm_rows(np.asarray(inputs["b_f"])).reshape(8, 128).T).astype(np.float32),
        "bih_b": np.ascontiguousarray(_perm_rows(np.asarray(inputs["b_b"])).reshape(8, 128).T).astype(np.float32),
        "woutT": np.ascontiguousarray(np.asarray(inputs["w_out"]).T.reshape(4, 128, K)).astype(dt_np),
        "bout": np.asarray(inputs["b_out"]).reshape(K, 1).astype(np.float32),
        "expAT": np.ascontiguousarray(np.exp(np.asarray(inputs["transition"], np.float64)).T).astype(np.float32),
        "trans": np.asarray(inputs["transition"], np.float32),
    }
    return m


def kernel(**inputs):
    import ml_dtypes
    from concourse import mybir
    from concourse.bass_utils import run_bass_kernel_spmd

    use_bf16 = _cache.get("use_bf16", True)
    key = ("prog", use_bf16)
    if key not in _cache:
        dt_w = mybir.dt.bfloat16 if use_bf16 else mybir.dt.float32
        _cache[key] = _build_program(dt_w)
    nc, names = _cache[key]
    dt_np = ml_dtypes.bfloat16 if use_bf16 else np.float32

    in_maps = []
    for k in range(NCORES):
        m = _prep_core(inputs, k, dt_np)
        in_maps.append({names[kk]: vv for kk, vv in m.items()})

    res = run_bass_kernel_spmd(nc, in_maps, core_ids=list(range(NCORES)),
                               **_cache.get("run_kwargs", {}))
    out = np.concatenate([r[names["out"]].reshape(BL) for r in res.results])
    _cache["last_results"] = res
    return out.astype(np.float32)


# revision 23
# speedup vs baseline: 2.4788x; 1.0267x over previous
"""BiLSTM-CRF loss kernel for Trainium2 (8 NeuronCores, data-parallel over batch).

v2 design (per core, B_loc=16 sequences):
  - Forward and backward LSTM directions run INTERLEAVED in a single
    512-iteration loop (iter i: fwd step t=i, bwd step t=511-i) so the two
    independent recurrence chains fill each other's engine stalls.
  - Gate order host-permuted to (i, f, o, g) so activations are 2 instrs
    per step: sigmoid over 96 cols + tanh over 32 cols.
  - xw window injected into the gate PSUM via an identity matmul
    (start=True) before the 16 W_hh matmuls accumulate on top; the ACT
    engine reads gates straight from PSUM (no separate gate-add).
  - c update is unmasked (pad mask is a suffix per sequence; the unfrozen
    c is never read back and stays bounded), h masked via copy_predicated.
  - h history per 32-step window -> batched emission matmuls (4/window/dir)
    instead of 2 per step.
  - All exp() for the CRF batched per window in the epilogue: keeps the
    sigmoid+tanh activation tables resident all of phase 1 (no
    ACT_TABLE_LOAD thrash).
  - CRF log-partition via backward beta recursion in exp space as a
    separate 511-step phase; rescaling folded into the next step's expE
    slice (off the critical path), ln() of the scales deferred to one
    batched instruction at the end.
  - Gold-path score (unary + transition gather) interleaved into the beta
    phase.
"""

import numpy as np

PAD_IDX = 0
VOCAB, K, E, H = 30000, 20, 256, 256
B, T = 128, 512
NCORES = 8
BL = B // NCORES          # 16 sequences per core
WIN = 32                  # proj window (time steps)
NW = T // WIN             # 16 windows
RESCALE = 8               # CRF rescale interval

_cache = {}


def _build_program(dt_w, ml=1):
    """Build the SPMD Bass program. dt_w: matmul weight/stream dtype.
    ml: min sequence length over the batch (all-active below this t)."""
    from contextlib import ExitStack
    import concourse.bass as bass
    import concourse.bacc as bacc
    import concourse.tile as tile
    from concourse import mybir
    from concourse.masks import make_identity

    f32 = mybir.dt.float32
    i32 = mybir.dt.int32

    nc = bacc.Bacc(None, target_bir_lowering=False, debug=False)
    names = {}

    with ExitStack() as ctx:
        tc = ctx.enter_context(tile.TileContext(nc))
        dram = ctx.enter_context(tc.tile_pool(name="dram", bufs=1, space="DRAM"))

        def din(key, shape, dt=f32):
            t = dram.tile(shape, dt, kind="ExternalInput", name=key)
            names[key] = t.tensor.name
            return t

        emb = din("emb", [VOCAB, E], dt_w)
        toks = din("toks", [T * BL, 1], i32)          # (w, j, b) window/j-major
        masku = din("masku", [1, T * BL], mybir.dt.uint8)  # col = t*16+b
        tags1h = din("tags1h", [K, T * BL], mybir.dt.uint8)  # one-hot(tag) * mask
        tagsnx = din("tagsnx", [T * BL, K])           # shifted one-hot * mask, f32
        tagsfl = din("tagsfl", [T * BL, 1], i32)      # tag ids, b-major
        wih = {d: din(f"wih_{d}", [E, 4 * H], dt_w) for d in "fb"}
        whh = {d: din(f"whh_{d}", [E, 4 * H], dt_w) for d in "fb"}
        bih = {d: din(f"bih_{d}", [128, 8]) for d in "fb"}
        woutT = din("woutT", [4, 128, K], dt_w)       # chunks: Fk0,Fk1,Bk0,Bk1
        bout = din("bout", [K, 1])
        expAT = din("expAT", [K, K])                  # exp(transition).T
        trans = din("trans", [K, K])                  # raw, for row gather
        out_loss = dram.tile([1, BL], f32, kind="ExternalOutput")
        names["out"] = out_loss.tensor.name

        sg = ctx.enter_context(tc.tile_pool(name="sg", bufs=1))       # singles
        tmp = ctx.enter_context(tc.tile_pool(name="tmp", bufs=3))     # step temps
        gat = ctx.enter_context(tc.tile_pool(name="gat", bufs=4))     # gather tiles
        winp = ctx.enter_context(tc.tile_pool(name="winp", bufs=2))   # xw windows
        hhp = ctx.enter_context(tc.tile_pool(name="hhp", bufs=2))     # h history
        xtw = ctx.enter_context(tc.tile_pool(name="xtw", bufs=2))
        fin = ctx.enter_context(tc.tile_pool(name="fin", bufs=3))     # finalize
        ps_g = ctx.enter_context(tc.tile_pool(name="ps_g", bufs=2, space="PSUM"))
        ps_w = ctx.enter_context(tc.tile_pool(name="ps_w", bufs=1, space="PSUM"))
        ps_e = ctx.enter_context(tc.tile_pool(name="ps_e", bufs=1, space="PSUM"))
        ps_s = ctx.enter_context(tc.tile_pool(name="ps_s", bufs=2, space="PSUM"))

        # ---- resident SBUF tensors ----
        s_wih = {d: sg.tile([128, 2, 4 * H], dt_w, tag=f"wih{d}", name=f"wih{d}") for d in "fb"}
        s_whh = {d: sg.tile([128, 2, 4 * H], dt_w, tag=f"whh{d}", name=f"whh{d}") for d in "fb"}
        for d in "fb":
            nc.sync.dma_start(out=s_wih[d][:], in_=wih[d][:].rearrange("(k p) m -> p k m", p=128))
            nc.sync.dma_start(out=s_whh[d][:], in_=whh[d][:].rearrange("(k p) m -> p k m", p=128))
        s_bih = {d: sg.tile([128, 8], f32, tag=f"bih{d}", name=f"bih{d}") for d in "fb"}
        for d in "fb":
            nc.sync.dma_start(out=s_bih[d][:], in_=bih[d][:])
        s_wout = sg.tile([128, 4, K], dt_w, tag="wout")
        nc.sync.dma_start(out=s_wout[:], in_=woutT[:].rearrange("c p k -> p c k"))
        s_bout = sg.tile([K, 1], f32, tag="bout")
        nc.sync.dma_start(out=s_bout[:], in_=bout[:])
        s_expAT = sg.tile([K, K], f32, tag="expAT")
        nc.sync.dma_start(out=s_expAT[:], in_=expAT[:])

        ones = sg.tile([128, K], f32, tag="ones")
        nc.vector.memset(ones[:], 1.0)
        identb = sg.tile([128, 128], dt_w, tag="identb")
        make_identity(nc, identb[:])

        # mask replica: (128, T, BL), col = t*16+b, broadcast across partitions
        maskrep = sg.tile([128, T, BL], mybir.dt.uint8, tag="maskrep")
        nc.sync.dma_start(
            out=maskrep[:],
            in_=bass.AP(tensor=masku.tensor, offset=masku[:].offset,
                        ap=[[0, 128], [BL, T], [1, BL]]),
        )

        emit = sg.tile([K, T, BL], f32, tag="emit")
        expE = sg.tile([K, T, BL], f32, tag="expE")

        # gather indices resident (one upfront DMA each)
        NT128 = T * BL // 128
        idxall = sg.tile([128, NT128], i32, tag="idxall")
        nc.sync.dma_start(out=idxall[:],
                          in_=bass.AP(tensor=toks.tensor, offset=toks[:].offset,
                                      ap=[[1, 128], [128, NT128]]))
        idxtag = sg.tile([128, NT128], i32, tag="idxtag")
        nc.sync.dma_start(out=idxtag[:],
                          in_=bass.AP(tensor=tagsfl.tensor, offset=tagsfl[:].offset,
                                      ap=[[1, 128], [128, NT128]]))
        s_t1h = sg.tile([K, T, BL], mybir.dt.uint8, tag="s_t1h")
        nc.sync.dma_start(out=s_t1h[:].rearrange("k t b -> k (t b)"), in_=tags1h[:])
        s_tnx = sg.tile([128, NT128, K], f32, tag="s_tnx")
        nc.sync.dma_start(out=s_tnx[:],
                          in_=tagsnx[:].rearrange("(n p) k -> p n k", p=128))

        # LSTM states (h in dt_w for matmul rhs, c in f32)
        st_h = {d: sg.tile([128, 2, BL], dt_w, tag=f"h{d}", name=f"h{d}") for d in "fb"}
        st_c = {d: sg.tile([128, 2, BL], f32, tag=f"c{d}", name=f"c{d}") for d in "fb"}
        for d in "fb":
            nc.vector.memset(st_h[d][:], 0.0)
            nc.vector.memset(st_c[d][:], 0.0)

        # CRF beta state (exp space) + deferred-ln scale buffer
        Bv = sg.tile([K, BL], f32, tag="Bv")
        nc.vector.memset(Bv[:], 1.0)
        NRS = T // RESCALE
        sums = sg.tile([1, NRS, BL], f32, tag="sums")
        nc.vector.memset(sums[:], 1.0)

        AF = mybir.ActivationFunctionType
        OP = mybir.AluOpType

        def mask_ap(t, parts, reps):
            """maskrep[:parts, t, :] replicated reps times along a middle dim."""
            base = maskrep[0:parts, t, :]
            if reps == 1:
                return base
            return bass.AP(tensor=base.tensor, offset=base.offset,
                           ap=[base.ap[0], [0, reps], [1, BL]])

        # warm-up matmuls: make PE's clock pass every weight-producing op so
        # steady-state matmuls carry at most one semaphore wait
        for wt in [s_wih["f"][:, 0, 0:1], s_wih["b"][:, 0, 0:1],
                   s_whh["f"][:, 0, 0:1], s_whh["b"][:, 0, 0:1],
                   s_wout[:, 0, 0:1], identb[:, 0:1]]:
            psd = ps_s.tile([1, 1], f32, tag="pssm")
            nc.tensor.matmul(psd[:], lhsT=wt, rhs=wt, start=True, stop=True)
        psd = ps_s.tile([1, 1], f32, tag="pssm")
        nc.tensor.matmul(psd[:], lhsT=s_expAT[0:K, 0:1], rhs=s_expAT[0:K, 0:1], start=True, stop=True)
        psd = ps_s.tile([1, 1], f32, tag="pssm")
        nc.tensor.matmul(psd[:], lhsT=ones[0:1, 0:1], rhs=ones[0:1, 0:1], start=True, stop=True)

        # ---------- window machinery ----------
        # win layout: (128, WIN, 8, BL) -> inject rhs win[:, j, :, :] is one
        # contiguous 128-col slice.  h_hist: (128, 2, WIN, BL) (k, j, b).
        cur_win = {}
        cur_hist = {}

        def build_window_thunks(d, tw):
            """Return (win_tile, thunk list) building xw window for t-window tw."""
            win = winp.tile([128, WIN, 8, BL], dt_w, tag=f"win{d}", name=f"win{d}")
            xT = xtw.tile([128, 2, 512], dt_w, tag=f"xT{d}", name=f"xT{d}")
            thunks = []
            pst_box = {}

            def gather(g):
                xg = gat.tile([128, E], dt_w, tag=f"xg{d}", name=f"xg{d}")
                nc.gpsimd.indirect_dma_start(
                    out=xg[:], out_offset=None, in_=emb[:],
                    in_offset=bass.IndirectOffsetOnAxis(ap=idxall[:, tw * 4 + g:tw * 4 + g + 1], axis=0),
                )
                pst_box[g] = xg

            def tp(g, k):
                xg = pst_box[g]
                pst = ps_s.tile([128, 128], dt_w, tag="pssm", name="pst")
                nc.tensor.transpose(out=pst[:], in_=xg[:, k * 128:(k + 1) * 128], identity=identb[:])
                nc.vector.tensor_copy(xT[:, k, g * 128:(g + 1) * 128], pst[:])

            for g in range(4):
                thunks.append(lambda g=g: gather(g))
                for k in range(2):
                    thunks.append(lambda g=g, k=k: tp(g, k))

            psw_box = {}

            def proj(m):
                psw = ps_w.tile([128, 512], f32, tag="psw", name="psw")
                for k in range(2):
                    nc.tensor.matmul(psw[:], lhsT=s_wih[d][:, k, m * 128:(m + 1) * 128],
                                     rhs=xT[:, k, :], start=(k == 0), stop=(k == 1))
                psw_box[m] = psw

            def bias_half(m, h):
                # psw cols are (j, b); win[:, jslice, m, :] free dims match
                nc.vector.tensor_scalar_add(win[:, 16 * h:16 * (h + 1), m, :],
                                            psw_box[m][:, 256 * h:256 * (h + 1)],
                                            s_bih[d][:, m:m + 1])

            for m in range(8):
                thunks.append(lambda m=m: proj(m))
                thunks.append(lambda m=m: bias_half(m, 0))
                thunks.append(lambda m=m: bias_half(m, 1))
            return win, thunks

        def new_hist(d):
            hist = hhp.tile([128, 2, WIN, BL], dt_w, tag=f"hist{d}", name=f"hist{d}")
            return hist

        def emit_window(d, tw, hist, first):
            """Batched emission for t-window tw from hist (ascending t slots)."""
            cbase = 0 if d == "f" else 2
            pse = ps_e.tile([K, 512], f32, tag="pse", name="pse")
            for k in range(2):
                nc.tensor.matmul(pse[:], lhsT=s_wout[:, cbase + k, :],
                                 rhs=hist[:, k, :, :], start=(k == 0), stop=(k == 1))
            dst = emit[:, tw * WIN:(tw + 1) * WIN, :].rearrange("k t b -> k (t b)")
            if first:
                nc.vector.tensor_scalar_add(dst[:], pse[:], s_bout[:, 0:1])
            else:
                nc.vector.tensor_tensor(dst[:], pse[:], dst[:], op=OP.add)

        # ---------- per-step pieces ----------
        # gate psum split (i,f,o) vs (g): the g matmuls are emitted first so
        # tanh(g) runs during the (i,f,o) matmul burst.
        def lstm_step(d, t, win, j):
            """One LSTM step for direction d at time t, window slot j."""
            psgg = ps_g.tile([128, 2, BL], f32, tag="psgg", name=f"psgg{d}")
            psgi = ps_g.tile([128, 6, BL], f32, tag="psgi", name=f"psgi{d}")
            h = st_h[d]
            nc.tensor.matmul(psgg[:].rearrange("p m b -> p (m b)"), lhsT=identb[:],
                             rhs=win[:, j, 6:8, :].rearrange("p m b -> p (m b)"),
                             start=True, stop=False, skip_group_check=True)
            for m in range(6, 8):
                for k in range(2):
                    nc.tensor.matmul(psgg[:, m - 6], lhsT=s_whh[d][:, k, m * 128:(m + 1) * 128],
                                     rhs=h[:, k, :], start=False, stop=(m == 7 and k == 1),
                                     skip_group_check=True)
            nc.tensor.matmul(psgi[:].rearrange("p m b -> p (m b)"), lhsT=identb[:],
                             rhs=win[:, j, 0:6, :].rearrange("p m b -> p (m b)"),
                             start=True, stop=False, skip_group_check=True)
            for m in range(6):
                for k in range(2):
                    nc.tensor.matmul(psgi[:, m], lhsT=s_whh[d][:, k, m * 128:(m + 1) * 128],
                                     rhs=h[:, k, :], start=False, stop=(m == 5 and k == 1),
                                     skip_group_check=True)
            return psgi, psgg

        def act_tanh_g(d, psgg):
            gg = tmp.tile([128, 2, BL], f32, tag=f"gg{d}", name=f"gg{d}")
            nc.scalar.activation(gg[:], psgg[:], AF.Tanh)
            return gg

        def act_sig(d, psgi):
            gates = tmp.tile([128, 6, BL], f32, tag=f"gates{d}", name=f"gates{d}")
            nc.scalar.activation(gates[:], psgi[:], AF.Sigmoid)
            return gates

        def cell_mults(d, gates, gg):
            # ig on gpsimd, fc on vector: the two products run on parallel engines
            ig = tmp.tile([128, 2, BL], f32, tag=f"ig{d}", name=f"ig{d}")
            nc.gpsimd.tensor_tensor(ig[:], gates[:, 0:2], gg[:], op=OP.mult)
            fc = tmp.tile([128, 2, BL], f32, tag=f"fc{d}", name=f"fc{d}")
            nc.vector.tensor_tensor(fc[:], gates[:, 2:4], st_c[d][:], op=OP.mult)
            return ig, fc

        def cell_update(d, ig, fc):
            # unmasked c update (frozen-region c is never read back)
            nc.vector.tensor_tensor(st_c[d][:], ig[:], fc[:], op=OP.add)

        def tanh_c(d):
            th = tmp.tile([128, 2, BL], f32, tag=f"th{d}", name=f"th{d}")
            nc.scalar.activation(th[:], st_c[d][:], AF.Tanh)
            return th

        # h_new is written straight into the hist slot (unmasked: frozen-slot
        # garbage is bounded and every consumer discards it); st_h is the only
        # masked state.
        def h_mult(d, gates, th, hist, j):
            nc.vector.tensor_tensor(hist[:, :, j, :], gates[:, 4:6], th[:], op=OP.mult)

        def h_state(d, t, hist, j):
            nc.vector.copy_predicated(st_h[d][:], mask_ap(t, 128, 2), hist[:, :, j, :])

        # ---------- prologue: build first windows ----------
        win_f, th_f = build_window_thunks("f", 0)
        for th in th_f:
            th()
        win_b, th_b = build_window_thunks("b", NW - 1)
        for th in th_b:
            th()
        cur_win["f"], cur_win["b"] = win_f, win_b
        cur_hist["f"], cur_hist["b"] = new_hist("f"), new_hist("b")
        prev_hist = {"f": None, "b": None}

        pending = []  # build thunks for next windows, drained ~2/iter

        # ---------- main interleaved loop ----------
        for i in range(T):
            blk, j = divmod(i, WIN)
            t_f = i
            t_b = T - 1 - i
            jb = WIN - 1 - j       # bwd hist slot (ascending t within window)

            if j == 0 and blk > 0:
                # windows blk-1 (fwd) and NW-blk (bwd t-window) just completed
                emit_window("f", blk - 1, prev_hist["f"], first=(blk - 1 <= 7))
                emit_window("b", NW - blk, prev_hist["b"], first=(NW - blk >= 8))

            # recurrence matmuls + activations, f then b staged
            psgi_f, psgg_f = lstm_step("f", t_f, cur_win["f"], j)
            psgi_b, psgg_b = lstm_step("b", t_b, cur_win["b"], jb)
            gg_f = act_tanh_g("f", psgg_f)
            g_f = act_sig("f", psgi_f)
            gg_b = act_tanh_g("b", psgg_b)
            ig_f, fc_f = cell_mults("f", g_f, gg_f)
            g_b = act_sig("b", psgi_b)
            cell_update("f", ig_f, fc_f)
            ig_b, fc_b = cell_mults("b", g_b, gg_b)
            th_ff = tanh_c("f")
            cell_update("b", ig_b, fc_b)
            th_bb = tanh_c("b")
            h_mult("f", g_f, th_ff, cur_hist["f"], j)
            h_state("f", t_f, cur_hist["f"], j)
            h_mult("b", g_b, th_bb, cur_hist["b"], jb)
            h_state("b", t_b, cur_hist["b"], jb)

            # window prefetch bookkeeping
            if j == 0:
                pending = []
                if blk + 1 < NW:
                    w2, tks = build_window_thunks("f", blk + 1)
                    nw_f = w2
                    pending.extend(tks)
                    w2, tks = build_window_thunks("b", NW - 2 - blk)
                    nw_b = w2
                    pending.extend(tks)
                    h2_f = new_hist("f")
                    h2_b = new_hist("b")
            # drain ~3 build thunks per iteration (90 per block)
            for _ in range(3):
                if pending:
                    pending.pop(0)()
            if j == WIN - 1:
                while pending:
                    pending.pop(0)()
                prev_hist["f"], prev_hist["b"] = cur_hist["f"], cur_hist["b"]
                if blk + 1 < NW:
                    cur_win["f"], cur_win["b"] = nw_f, nw_b
                    cur_hist["f"], cur_hist["b"] = h2_f, h2_b

        # ---------- epilogue: last emissions + batched exp ----------
        emit_window("f", NW - 1, prev_hist["f"], first=False)
        emit_window("b", 0, prev_hist["b"], first=False)
        for tw in range(NW - 1, -1, -1):
            src = emit[:, tw * WIN:(tw + 1) * WIN, :].rearrange("k t b -> k (t b)")
            dst = expE[:, tw * WIN:(tw + 1) * WIN, :].rearrange("k t b -> k (t b)")
            nc.scalar.activation(dst[:], src[:], AF.Exp)

        # ---------- finalize thunks (interleaved into beta phase) ----------
        fin_thunks = []
        Uacc = fin.tile([K, BL], f32, tag="Uacc")
        nc.vector.memset(Uacc[:], 0.0)
        CH = 32
        TC = T // CH

        def unary_chunk(ci):
            t1 = fin.tile([K, TC * BL], f32, tag="t1")
            nc.vector.tensor_copy(t1[:], s_t1h[:, ci * TC:(ci + 1) * TC, :].rearrange("p t b -> p (t b)"))
            um = fin.tile([K, TC * BL], f32, tag="um")
            nc.vector.tensor_tensor(
                um[:], t1[:],
                emit[:, ci * TC:(ci + 1) * TC, :].rearrange("p t b -> p (t b)"),
                op=OP.mult)
            ur = fin.tile([K, BL], f32, tag="ur")
            umr = bass.AP(tensor=um.tensor, offset=um[:].offset,
                          ap=[um[:].ap[0], [1, BL], [BL, TC]])
            nc.vector.tensor_reduce(ur[:], umr, axis=mybir.AxisListType.X, op=OP.add)
            nc.vector.tensor_tensor(Uacc[:], Uacc[:], ur[:], op=OP.add)

        for ci in range(CH):
            fin_thunks.append(lambda ci=ci: unary_chunk(ci))

        TRbuf = fin.tile([128, NT128], f32, tag="TRbuf")

        def trans_chunk(i):
            tr = gat.tile([128, K], f32, tag="tr")
            nc.gpsimd.indirect_dma_start(
                out=tr[:], out_offset=None, in_=trans[:],
                in_offset=bass.IndirectOffsetOnAxis(ap=idxtag[:, i:i + 1], axis=0))
            nc.vector.tensor_tensor(tr[:], tr[:], s_tnx[:, i, :], op=OP.mult)
            nc.vector.tensor_reduce(TRbuf[:, i:i + 1], tr[:], axis=mybir.AxisListType.X, op=OP.add)

        for i in range(NT128):
            fin_thunks.append(lambda i=i: trans_chunk(i))

        # ---------- CRF beta recursion (2 independent batch strands) ----------
        # For tstep+1 < ml every sequence is active: the copy_predicated
        # vanishes and the recursion state stays in PSUM (2-stage chain).
        NS = 2
        SB = BL // NS

        def scol(s):
            return slice(s * SB, (s + 1) * SB)

        cur_psb = [None] * NS   # PSUM-carried state per strand (all-active phase)

        def bv_src(s):
            return cur_psb[s][:] if cur_psb[s] is not None else Bv[:, scol(s)]

        for tstep in range(T - 2, -1, -1):
            tp1 = tstep + 1
            allact = tp1 < ml
            bps = []
            for s in range(NS):
                bp = tmp.tile([K, SB], f32, tag=f"bp{s}", name=f"bp{s}")
                nc.vector.tensor_tensor(bp[:], bv_src(s), expE[:, tp1, scol(s)], op=OP.mult)
                bps.append(bp)
            psbs = []
            for s in range(NS):
                psb = ps_s.tile([K, SB], f32, tag="pssm", name=f"psb{s}")
                nc.tensor.matmul(psb[:], lhsT=s_expAT[:], rhs=bps[s][:], start=True, stop=True)
                psbs.append(psb)
            need_sbuf = (tstep % RESCALE == 0 and tstep > 0) or tstep == 0
            for s in range(NS):
                if allact:
                    cur_psb[s] = psbs[s]
                    if need_sbuf:
                        nc.vector.tensor_copy(Bv[:, scol(s)], psbs[s][:])
                else:
                    nc.vector.copy_predicated(Bv[:, scol(s)], maskrep[0:K, tp1, scol(s)], psbs[s][:])

            if tstep % RESCALE == 0 and tstep > 0:
                ri = tstep // RESCALE
                for s in range(NS):
                    pss = ps_w.tile([1, SB], f32, tag="psw", name=f"pss{s}")
                    nc.tensor.matmul(pss[:], lhsT=ones[0:K, 0:1], rhs=Bv[:, scol(s)], start=True, stop=True)
                    if allact:
                        nc.vector.tensor_copy(sums[:, ri, scol(s)], pss[:])
                    else:
                        nc.vector.copy_predicated(sums[:, ri, scol(s)], maskrep[0:1, tstep, scol(s)], pss[:])
                    rr = tmp.tile([1, SB], f32, tag=f"rr{s}")
                    nc.vector.reciprocal(rr[:], pss[:])
                    psr = ps_e.tile([K, SB], f32, tag="pse", name=f"psr{s}")
                    nc.tensor.matmul(psr[:], lhsT=ones[0:1, 0:K], rhs=rr[:], start=True, stop=True)
                    # fold the rescale into the expE slice the next step consumes
                    nc.vector.tensor_tensor(expE[:, tstep, scol(s)], expE[:, tstep, scol(s)],
                                            psr[:], op=OP.mult)

            if fin_thunks and tstep % 5 == 0:
                fin_thunks.pop(0)()

        while fin_thunks:
            fin_thunks.pop(0)()

        # ---------- final assembly ----------
        zt = fin.tile([K, BL], f32, tag="zt")
        nc.vector.tensor_tensor(zt[:], Bv[:], expE[:, 0, :], op=OP.mult)
        psz = ps_s.tile([1, BL], f32, tag="pssm")
        nc.tensor.matmul(psz[:], lhsT=ones[0:K, 0:1], rhs=zt[:], start=True, stop=True)
        logZ = fin.tile([1, BL], f32, tag="logZ")
        nc.scalar.activation(logZ[:], psz[:], AF.Ln)

        # deferred ln of the rescale sums: one batched Ln + strided reduce
        lns = fin.tile([1, NRS, BL], f32, tag="lns")
        nc.scalar.activation(lns[:].rearrange("p r b -> p (r b)"),
                             sums[:].rearrange("p r b -> p (r b)"), AF.Ln)
        lsum = fin.tile([1, BL], f32, tag="lsum")
        lns_ap = bass.AP(tensor=lns.tensor, offset=lns[:].offset,
                         ap=[lns[:].ap[0], [1, BL], [BL, NRS]])
        nc.vector.tensor_reduce(lsum[:], lns_ap, axis=mybir.AxisListType.X, op=OP.add)
        nc.vector.tensor_tensor(logZ[:], logZ[:], lsum[:], op=OP.add)

        # unary total
        psu = ps_s.tile([1, BL], f32, tag="pssm")
        nc.tensor.matmul(psu[:], lhsT=ones[0:K, 0:1], rhs=Uacc[:], start=True, stop=True)
        score = fin.tile([1, BL], f32, tag="score")
        nc.vector.tensor_copy(score[:], psu[:])

        # transition total: colsum TRbuf then per-b strided reduce
        QT = T // 128
        pstr = ps_s.tile([1, NT128], f32, tag="pssm")
        nc.tensor.matmul(pstr[:], lhsT=ones[:, 0:1], rhs=TRbuf[:], start=True, stop=True)
        trv = fin.tile([1, BL], f32, tag="trv")
        ptr_ap = bass.AP(tensor=pstr.tensor, offset=pstr[:].offset,
                         ap=[pstr[:].ap[0], [QT, BL], [1, QT]])
        nc.vector.tensor_reduce(trv[:], ptr_ap, axis=mybir.AxisListType.X, op=OP.add)

        # loss = logZ - (score + trans)
        nc.vector.tensor_tensor(score[:], score[:], trv[:], op=OP.add)
        res = fin.tile([1, BL], f32, tag="res")
        nc.vector.tensor_tensor(res[:], logZ[:], score[:], op=OP.subtract)
        nc.sync.dma_start(out=out_loss[:], in_=res[:])

    nc.compile()
    return nc, names


# torch gate order (i, f, g, o) -> kernel order (i, f, o, g)
def _perm_rows(w):
    return np.concatenate([w[0:2 * H], w[3 * H:4 * H], w[2 * H:3 * H]], axis=0)


def _prep_core(inputs, k, dt_np):
    """Build the per-core input map (host-side index plumbing only)."""
    s = slice(k * BL, (k + 1) * BL)
    sent = np.asarray(inputs["sentences"][s])          # (16, 512) i32
    tags = np.asarray(inputs["tags"][s])               # (16, 512) i32
    mask = (sent != PAD_IDX)
    # toks in (w, j, b) order so gathered/psw columns are (j, b)
    toks = sent.reshape(BL, NW, WIN).transpose(1, 2, 0).reshape(T * BL, 1)
    oh = (tags[:, :, None] == np.arange(K)[None, None, :])
    tags1h = (oh & mask[:, :, None]).transpose(2, 1, 0).reshape(K, T * BL)
    tnx = np.zeros((BL, T, K), np.float32)
    tnx[:, :-1, :] = (oh[:, 1:, :] & mask[:, 1:, None]).astype(np.float32)
    m = {
        "toks": toks.astype(np.int32),
        "masku": mask.T.astype(np.uint8).reshape(1, T * BL),
        "tags1h": tags1h.astype(np.uint8),
        "tagsnx": tnx.reshape(T * BL, K).astype(np.float32),
        "tagsfl": tags.reshape(T * BL, 1).astype(np.int32),
        "emb": np.asarray(inputs["embedding"]).astype(dt_np),
        "wih_f": np.ascontiguousarray(_perm_rows(np.asarray(inputs["w_ih_f"])).T).astype(dt_np),
        "wih_b": np.ascontiguousarray(_perm_rows(np.asarray(inputs["w_ih_b"])).T).astype(dt_np),
        "whh_f": np.ascontiguousarray(_perm_rows(np.asarray(inputs["w_hh_f"])).T).astype(dt_np),
        "whh_b": np.ascontiguousarray(_perm_rows(np.asarray(inputs["w_hh_b"])).T).astype(dt_np),
        "bih_f": np.ascontiguousarray(_perm_rows(np.asarray(inputs["b_f"])).reshape(8, 128).T).astype(np.float32),
        "bih_b": np.ascontiguousarray(_perm_rows(np.asarray(inputs["b_b"])).reshape(8, 128).T).astype(np.float32),
        "woutT": np.ascontiguousarray(np.asarray(inputs["w_out"]).T.reshape(4, 128, K)).astype(dt_np),
        "bout": np.asarray(inputs["b_out"]).reshape(K, 1).astype(np.float32),
        "expAT": np.ascontiguousarray(np.exp(np.asarray(inputs["transition"], np.float64)).T).astype(np.float32),
        "trans": np.asarray(inputs["transition"], np.float32),
    }
    return m


def kernel(**inputs):
    import ml_dtypes
    from concourse import mybir
    from concourse.bass_utils import run_bass_kernel_spmd

    use_bf16 = _cache.get("use_bf16", True)
    ml = max(1, int(np.asarray(inputs["sentences_lengths"]).min()))
    key = ("prog", use_bf16, ml)
    if key not in _cache:
        dt_w = mybir.dt.bfloat16 if use_bf16 else mybir.dt.float32
        _cache[key] = _build_program(dt_w, ml)
    nc, names = _cache[key]
    dt_np = ml_dtypes.bfloat16 if use_bf16 else np.float32

    in_maps = []
    for k in range(NCORES):
        m = _prep_core(inputs, k, dt_np)
        in_maps.append({names[kk]: vv for kk, vv in m.items()})

    res = run_bass_kernel_spmd(nc, in_maps, core_ids=list(range(NCORES)),
                               **_cache.get("run_kwargs", {}))
    out = np.concatenate([r[names["out"]].reshape(BL) for r in res.results])
    _cache["last_results"] = res
    return out.astype(np.float32)
